# revision 1
# baseline (speedup 1.0000x reference)
"""SchNet forward on 8 Trainium2 NeuronCores (Bass/Tile), data-parallel over molecules.

kernel(**inputs) takes FULL inputs (as produced by setup_inputs) and returns
the FULL [256] float32 per-molecule energies. Inside: shards 256 molecules
into 8 groups of 32 (1024 atoms each), runs an SPMD Bass kernel on cores 0-7,
gathers outputs.

Hardcoded shape: N=8192 atoms, 32 atoms/molecule, FEAT=100, NG=25, K=28, L=4,
CUTOFF=6.  Per core: 1024 atoms, all-pairs 32x32 block distances (E=32768
edge slots); top-28 selection done by rank counting; non-selected edges get
distance=CUTOFF so the cosine cutoff zeroes them exactly like the reference's
top_k + ccut weighting.
"""

import math
import numpy as np

N = 8192
APM = 32
FEAT = 100
NG = 25
K = 28
L = 4
CUTOFF = 6.0
NCORES = 8
NA = N // NCORES          # atoms per core = 1024
NM = NA // APM            # molecules per core = 32
E = NA * APM              # edge slots per core = 32768
EG = E // 4               # edges per partition-group = 8192
EC = 1024                 # edge chunk = one molecule's 32x32 pairs
H = FEAT // 2
NBLK = NA // 128          # 8 atom blocks per core

_COMPILED = None


def _build(repeats: int = 1):
    import concourse.bass as bass
    import concourse.mybir as mybir
    import concourse.tile as tile
    from concourse import bacc

    dt = mybir.dt
    F32 = dt.float32
    F32R = dt.float32r
    A = mybir.ActivationFunctionType
    OP = mybir.AluOpType
    AX = mybir.AxisListType
    LF = L * FEAT

    nc = bacc.Bacc()

    pos_d = nc.dram_tensor("pos", [NA, 3], F32, kind="ExternalInput")
    h0_d = nc.dram_tensor("h0", [FEAT, NA], F32, kind="ExternalInput")
    w1rep_d = nc.dram_tensor("w1rep", [L, 128, FEAT], F32, kind="ExternalInput")
    w2_d = nc.dram_tensor("w2", [L, FEAT, FEAT], F32, kind="ExternalInput")
    b1_d = nc.dram_tensor("b1", [L, FEAT], F32, kind="ExternalInput")
    b2_d = nc.dram_tensor("b2", [L, FEAT], F32, kind="ExternalInput")
    l1w_d = nc.dram_tensor("l1w", [L, FEAT, FEAT], F32, kind="ExternalInput")
    l2w_d = nc.dram_tensor("l2w", [L, FEAT, FEAT], F32, kind="ExternalInput")
    l2b_d = nc.dram_tensor("l2b", [L, FEAT], F32, kind="ExternalInput")
    lw_d = nc.dram_tensor("lw", [L, FEAT, FEAT], F32, kind="ExternalInput")
    lb_d = nc.dram_tensor("lb", [L, FEAT], F32, kind="ExternalInput")
    ow1_d = nc.dram_tensor("ow1", [FEAT, H], F32, kind="ExternalInput")
    ob1_d = nc.dram_tensor("ob1", [H], F32, kind="ExternalInput")
    ow2_d = nc.dram_tensor("ow2", [H, 1], F32, kind="ExternalInput")
    ob2_d = nc.dram_tensor("ob2", [1], F32, kind="ExternalInput")
    diag_d = nc.dram_tensor("diagc", [128, APM], F32, kind="ExternalInput")
    offs_d = nc.dram_tensor("offs", [128, 1], F32, kind="ExternalInput")

    out_d = nc.dram_tensor("energy", [NM], F32, kind="ExternalOutput")

    dtil_dram = nc.dram_tensor("dtil_lin", [E], F32)
    gam_dram = nc.dram_tensor("gam_lin", [E], F32R)

    def bap(a, off, dims):
        return bass.AP(tensor=a.tensor, offset=a.offset + off, ap=dims)

    with tile.TileContext(nc) as tc:
        import contextlib
        ctx = contextlib.ExitStack()
        with ctx:
            persist = ctx.enter_context(tc.tile_pool(name="persist", bufs=1))
            wpool = ctx.enter_context(tc.tile_pool(name="weights", bufs=1))
            psA = ctx.enter_context(tc.tile_pool(name="psA", bufs=2, space="PSUM"))
            psB = ctx.enter_context(tc.tile_pool(name="psB", bufs=2, space="PSUM"))

            # persistent tiles
            ea0 = persist.tile([128, EG], F32R, tag="ea0")   # groups 0(base0),1(base64)
            ea1 = persist.tile([128, EG], F32R, tag="ea1")   # groups 2(base0),3(base64)
            hA = persist.tile([FEAT, NA], F32, tag="hA")
            hB = persist.tile([FEAT, NA], F32, tag="hB")
            x1_t = persist.tile([FEAT, NA], F32, tag="x1")
            agg_t = persist.tile([FEAT, NA], F32, tag="agg")
            half_t = persist.tile([128, 1], F32, tag="half")
            nhalfpi_t = persist.tile([128, 1], F32, tag="nhalfpi")
            diag_t = persist.tile([128, APM], F32, tag="diag")
            offs_t = persist.tile([128, 1], F32, tag="offs")
            nc.vector.memset(half_t[:], 0.5)
            nc.vector.memset(nhalfpi_t[:], -math.pi / 2)
            nc.sync.dma_start(out=diag_t[:], in_=diag_d[:])
            nc.sync.dma_start(out=offs_t[:], in_=offs_d[:])

            # weights
            w1f = wpool.tile([128, LF], F32, tag="w1f")
            w1_t = wpool.tile([128, LF], F32R, tag="w1")
            w2f = wpool.tile([FEAT, LF], F32, tag="w2f")
            w2_t = wpool.tile([FEAT, LF], F32R, tag="w2")
            b2f = wpool.tile([128, LF], F32, tag="b2f")
            b2r_t = wpool.tile([128, LF], F32R, tag="b2r")   # row 64 holds b2 per layer
            l1w_t = wpool.tile([FEAT, LF], F32, tag="l1w")
            l2w_t = wpool.tile([FEAT, LF], F32, tag="l2w")
            lw_t = wpool.tile([FEAT, LF], F32, tag="lww")
            b1_t = wpool.tile([FEAT, L], F32, tag="b1")
            l2b_t = wpool.tile([FEAT, L], F32, tag="l2b")
            lb_t = wpool.tile([FEAT, L], F32, tag="lb")
            ow1_t = wpool.tile([FEAT, H], F32, tag="ow1")
            ob1_t = wpool.tile([H, 1], F32, tag="ob1")
            ow2_t = wpool.tile([H, 1], F32, tag="ow2")
            ob2_t = wpool.tile([1, 1], F32, tag="ob2")

            nc.sync.dma_start(out=w1f[:].rearrange("p (l f) -> p l f", f=FEAT),
                              in_=w1rep_d[:].transpose([1, 0, 2]))
            nc.vector.tensor_copy(w1_t[:], w1f[:])
            nc.sync.dma_start(out=w2f[:].rearrange("p (l f) -> p l f", f=FEAT),
                              in_=w2_d[:].transpose([1, 0, 2]))
            nc.vector.tensor_copy(w2_t[:], w2f[:])
            nc.vector.memset(b2f[:], 0.0)
            nc.sync.dma_start(
                out=b2f[64:65, :].rearrange("p (l f) -> p l f", f=FEAT),
                in_=b2_d[:].unsqueeze(0))
            nc.vector.tensor_copy(b2r_t[:], b2f[:])
            nc.sync.dma_start(out=l1w_t[:].rearrange("p (l f) -> p l f", f=FEAT),
                              in_=l1w_d[:].transpose([1, 0, 2]))
            nc.sync.dma_start(out=l2w_t[:].rearrange("p (l f) -> p l f", f=FEAT),
                              in_=l2w_d[:].transpose([1, 0, 2]))
            nc.sync.dma_start(out=lw_t[:].rearrange("p (l f) -> p l f", f=FEAT),
                              in_=lw_d[:].transpose([1, 0, 2]))
            nc.sync.dma_start(out=b1_t[:], in_=b1_d[:].transpose([1, 0]))
            nc.sync.dma_start(out=l2b_t[:], in_=l2b_d[:].transpose([1, 0]))
            nc.sync.dma_start(out=lb_t[:], in_=lb_d[:].transpose([1, 0]))
            nc.sync.dma_start(out=ow1_t[:], in_=ow1_d[:])
            nc.sync.dma_start(out=ob1_t[:], in_=ob1_d[:].unsqueeze(1))
            nc.sync.dma_start(out=ow2_t[:], in_=ow2_d[:])
            nc.sync.dma_start(out=ob2_t[:], in_=ob2_d[:].unsqueeze(1))

            for rep in range(repeats):
                nc.sync.dma_start(out=hA[:], in_=h0_d[:])
                pA = tc.tile_pool(name=f"bld{rep}", bufs=1)
                pAs = tc.tile_pool(name=f"scrA{rep}", bufs=2)
                with pA as bp, pAs as sc:
                    # ========== PHASE A: graph build ==========
                    EA_ = NBLK * APM
                    d2all = bp.tile([128, EA_], F32, tag="d2all")
                    for b in range(NBLK):
                        posP = sc.tile([128, 3], F32, tag="posP")
                        nc.sync.dma_start(out=posP[:], in_=pos_d[128 * b:128 * (b + 1), :])
                        posB = sc.tile([128, APM, 3], F32, tag="posB")
                        nc.sync.dma_start(
                            out=posB[:],
                            in_=bap(pos_d[:], 4 * b * APM * 3,
                                    [[APM * 3, 4], [0, APM], [3, APM], [1, 3]]))
                        dif = sc.tile([128, APM, 3], F32, tag="dif")
                        pP = posP[:]
                        nc.vector.tensor_tensor(
                            out=dif[:],
                            in0=bap(pP, 0, [pP.ap[0], [0, APM], [1, 3]]),
                            in1=posB[:], op=OP.subtract)
                        sq = sc.tile([128, APM, 3], F32, tag="sq")
                        nc.vector.tensor_tensor(out=sq[:], in0=dif[:], in1=dif[:],
                                                op=OP.mult)
                        nc.vector.tensor_reduce(out=d2all[:, APM * b:APM * (b + 1)],
                                                in_=sq[:], axis=AX.X, op=OP.add)
                    gtm = bp.tile([128, EA_], F32, tag="gtm")
                    nc.vector.tensor_scalar(out=gtm[:], in0=d2all[:], scalar1=36.0,
                                            scalar2=None, op0=OP.is_gt)
                    mask = bp.tile([128, EA_], F32, tag="mask")
                    for b in range(NBLK):
                        nc.vector.tensor_tensor(out=mask[:, APM * b:APM * (b + 1)],
                                                in0=gtm[:, APM * b:APM * (b + 1)],
                                                in1=diag_t[:], op=OP.max)
                    inv = bp.tile([128, EA_], F32, tag="inv")
                    nc.vector.tensor_scalar(out=inv[:], in0=mask[:], scalar1=-1.0,
                                            scalar2=1.0, op0=OP.mult, op1=OP.add)
                    d2m = bp.tile([128, EA_], F32, tag="d2m")
                    nc.vector.tensor_tensor(out=d2m[:], in0=d2all[:], in1=inv[:],
                                            op=OP.mult)
                    m36 = bp.tile([128, EA_], F32, tag="m36")
                    nc.vector.tensor_scalar(out=m36[:], in0=mask[:], scalar1=36.0,
                                            scalar2=None, op0=OP.mult)
                    nc.vector.tensor_tensor(out=d2m[:], in0=d2m[:], in1=m36[:],
                                            op=OP.add)

                    sel = bp.tile([128, EA_], F32, tag="sel")
                    for b in range(NBLK):
                        dd = d2m[:, APM * b:APM * (b + 1)]
                        lt = sc.tile([128, APM, APM], F32, tag="lt")
                        nc.vector.tensor_tensor(
                            out=lt[:],
                            in0=bap(dd, 0, [dd.ap[0], [0, APM], [1, APM]]),
                            in1=bap(dd, 0, [dd.ap[0], [1, APM], [0, APM]]),
                            op=OP.is_lt)
                        rk = sc.tile([128, APM], F32, tag="rk")
                        nc.vector.tensor_reduce(out=rk[:], in_=lt[:], axis=AX.X,
                                                op=OP.add)
                        nc.vector.tensor_scalar(out=sel[:, APM * b:APM * (b + 1)],
                                                in0=rk[:], scalar1=float(K) - 0.5,
                                                scalar2=None, op0=OP.is_lt)

                    s_t = bp.tile([128, EA_], F32, tag="s_t")
                    nc.scalar.activation(s_t[:], d2m[:], A.Sqrt)
                    for _ in range(2):
                        rc = sc.tile([128, EA_], F32, tag="rc")
                        nc.vector.reciprocal(rc[:], s_t[:])
                        tq = sc.tile([128, EA_], F32, tag="tq")
                        nc.vector.tensor_tensor(out=tq[:], in0=d2m[:], in1=rc[:],
                                                op=OP.mult)
                        nc.vector.tensor_tensor(out=s_t[:], in0=s_t[:], in1=tq[:],
                                                op=OP.add)
                        nc.vector.tensor_scalar(out=s_t[:], in0=s_t[:], scalar1=0.5,
                                                scalar2=None, op0=OP.mult)
                    dm6 = bp.tile([128, EA_], F32, tag="dm6")
                    nc.vector.tensor_scalar(out=dm6[:], in0=s_t[:], scalar1=-6.0,
                                            scalar2=None, op0=OP.add)
                    dtil = bp.tile([128, EA_], F32, tag="dtil")
                    nc.vector.tensor_tensor(out=dtil[:], in0=sel[:], in1=dm6[:],
                                            op=OP.mult)
                    nc.vector.tensor_scalar(out=dtil[:], in0=dtil[:], scalar1=6.0,
                                            scalar2=None, op0=OP.add)
                    sn = bp.tile([128, EA_], F32, tag="sn")
                    nc.scalar.activation(sn[:], dtil[:], A.Sin, bias=nhalfpi_t[:],
                                         scale=float(math.pi / 6.0))
                    nc.vector.tensor_scalar(out=sn[:], in0=sn[:], scalar1=-0.5,
                                            scalar2=0.5, op0=OP.mult, op1=OP.add)
                    ilt = bp.tile([128, EA_], F32, tag="ilt")
                    nc.vector.tensor_scalar(out=ilt[:], in0=d2m[:], scalar1=36.0,
                                            scalar2=None, op0=OP.is_lt)
                    nc.vector.tensor_tensor(out=ilt[:], in0=ilt[:], in1=sel[:],
                                            op=OP.mult)
                    gam = bp.tile([128, EA_], F32R, tag="gam")
                    nc.vector.tensor_tensor(out=gam[:], in0=sn[:], in1=ilt[:],
                                            op=OP.mult)

                    for b in range(NBLK):
                        nc.sync.dma_start(
                            out=bap(dtil_dram[:], 4096 * b, [[APM, 128], [1, APM]]),
                            in_=dtil[:, APM * b:APM * (b + 1)])
                        nc.sync.dma_start(
                            out=bap(gam_dram[:], 4096 * b, [[APM, 128], [1, APM]]),
                            in_=gam[:, APM * b:APM * (b + 1)])

                    # drep tiles: tile t holds groups {2t,2t+1} at bases {0,64}
                    for t_i, ea_tile in ((0, ea0), (1, ea1)):
                        drep = bp.tile([128, EG], F32, tag="drep")
                        for gg in range(2):
                            g = 2 * t_i + gg
                            dst = bap(drep[:], 0,
                                      [[drep[:].ap[0][0] * 64, 1],
                                       [drep[:].ap[0][0], 32], [1, EG]])
                            dst = bass.AP(tensor=drep[:].tensor,
                                          offset=drep[:].offset,
                                          ap=[[drep[:].ap[0][0], 32], [1, EG]]) \
                                if gg == 0 else \
                                bass.AP(tensor=drep[:].tensor,
                                        offset=drep[:].offset + 64 * drep[:].ap[0][0],
                                        ap=[[drep[:].ap[0][0], 32], [1, EG]])
                            nc.sync.dma_start(
                                out=dst,
                                in_=bap(dtil_dram[:], EG * g, [[0, 32], [1, EG]]))
                        for cj in range(EG // 2048):
                            ssl = slice(2048 * cj, 2048 * (cj + 1))
                            q = sc.tile([128, 2048], F32, tag="q")
                            nc.vector.tensor_scalar(out=q[:], in0=drep[:, ssl],
                                                    scalar1=offs_t[:], scalar2=None,
                                                    op0=OP.subtract)
                            nc.vector.tensor_tensor(out=q[:], in0=q[:], in1=q[:],
                                                    op=OP.mult)
                            nc.scalar.activation(ea_tile[:, ssl], q[:], A.Exp,
                                                 scale=-8.0)

                with tc.tile_pool(name=f"scrB{rep}", bufs=2) as sc:
                    # ========== PHASE B: interaction layers ==========
                    hcur, hnxt = hA, hB
                    for l in range(L):
                        lf = slice(FEAT * l, FEAT * (l + 1))
                        ps_n = psA.tile([FEAT, NA], F32, tag="psA")
                        for hh in range(2):
                            qs = slice(512 * hh, 512 * (hh + 1))
                            nc.tensor.matmul(ps_n[:, qs], l1w_t[:, lf], hcur[:, qs],
                                             start=True, stop=True)
                        nc.vector.tensor_copy(x1_t[:], ps_n[:])

                        for ci in range(E // EC):
                            g, cj = divmod(ci, EG // EC)
                            ea_tile = ea0 if g < 2 else ea1
                            base = 64 * (g % 2)
                            ps1 = psA.tile([FEAT, EC], F32, tag="psA")
                            for q2 in range(EC // 512):
                                qs = slice(512 * q2, 512 * (q2 + 1))
                                nc.tensor.matmul(
                                    ps1[:, qs],
                                    w1_t[base:base + NG, lf],
                                    ea_tile[base:base + NG,
                                            EC * cj + 512 * q2:EC * cj + 512 * (q2 + 1)],
                                    start=True, stop=True)
                            ue = sc.tile([FEAT, EC], F32, tag="ue")
                            nc.scalar.activation(ue[:], ps1[:], A.Exp,
                                                 bias=b1_t[:, l:l + 1])
                            u = sc.tile([FEAT, EC], F32, tag="u")
                            nc.scalar.activation(u[:], ue[:], A.Ln,
                                                 bias=half_t[:FEAT], scale=0.5)
                            gr = sc.tile([128, EC], F32R, tag="gr")
                            nc.sync.dma_start(
                                out=gr[:],
                                in_=bap(gam_dram[:], EG * g + EC * cj,
                                        [[0, 128], [1, EC]]))
                            up = sc.tile([FEAT, EC], F32R, tag="up")
                            nc.vector.tensor_tensor(out=up[:], in0=u[:],
                                                    in1=gr[:FEAT, :], op=OP.mult)
                            ps2 = psB.tile([FEAT, EC], F32, tag="psB")
                            for q2 in range(EC // 512):
                                qs = slice(512 * q2, 512 * (q2 + 1))
                                nc.tensor.matmul(ps2[:, qs], w2_t[:, lf], up[:, qs],
                                                 start=True, stop=False)
                                nc.tensor.matmul(ps2[:, qs], b2r_t[64:65, lf],
                                                 gr[64:65, qs], start=False, stop=True)
                            a0 = 256 * g + 32 * cj   # first atom of this molecule
                            x1b = x1_t[:]
                            msg = sc.tile([FEAT, EC], F32, tag="msg")
                            nc.vector.tensor_tensor(
                                out=msg[:], in0=ps2[:],
                                in1=bap(x1b, a0, [x1b.ap[0], [0, APM], [1, APM]]),
                                op=OP.mult)
                            nc.vector.tensor_reduce(
                                out=agg_t[:, a0:a0 + APM],
                                in_=msg[:].rearrange("p (a j) -> p a j", j=APM),
                                axis=AX.X, op=OP.add)

                        ps_v = psA.tile([FEAT, NA], F32, tag="psA")
                        for hh in range(2):
                            qs = slice(512 * hh, 512 * (hh + 1))
                            nc.tensor.matmul(ps_v[:, qs], l2w_t[:, lf], agg_t[:, qs],
                                             start=True, stop=True)
                        spe = sc.tile([FEAT, NA], F32, tag="ue")
                        nc.scalar.activation(spe[:], ps_v[:], A.Exp,
                                             bias=l2b_t[:, l:l + 1])
                        spl = sc.tile([FEAT, NA], F32, tag="u")
                        nc.scalar.activation(spl[:], spe[:], A.Ln,
                                             bias=half_t[:FEAT], scale=0.5)
                        ps_w = psB.tile([FEAT, NA], F32, tag="psB")
                        for hh in range(2):
                            qs = slice(512 * hh, 512 * (hh + 1))
                            nc.tensor.matmul(ps_w[:, qs], lw_t[:, lf], spl[:, qs],
                                             start=True, stop=True)
                        nc.vector.scalar_tensor_tensor(
                            out=hnxt[:], in0=ps_w[:], scalar=lb_t[:, l:l + 1],
                            in1=hcur[:], op0=OP.add, op1=OP.add)
                        hcur, hnxt = hnxt, hcur

                    # ========== PHASE C: readout ==========
                    ps_r = psA.tile([FEAT, NA], F32, tag="psA")
                    for hh in range(2):
                        qs = slice(512 * hh, 512 * (hh + 1))
                        nc.tensor.matmul(ps_r[:H, qs], ow1_t[:], hcur[:, qs],
                                         start=True, stop=True)
                    re = sc.tile([H, NA], F32, tag="ue")
                    nc.scalar.activation(re[:], ps_r[:H, :], A.Exp, bias=ob1_t[:])
                    rl = sc.tile([H, NA], F32, tag="u")
                    nc.scalar.activation(rl[:], re[:], A.Ln, bias=half_t[:H],
                                         scale=0.5)
                    ps_e = psB.tile([FEAT, NA], F32, tag="psB")
                    for hh in range(2):
                        qs = slice(512 * hh, 512 * (hh + 1))
                        nc.tensor.matmul(ps_e[:1, qs], ow2_t[:], rl[:, qs],
                                         start=True, stop=True)
                    pa = sc.tile([1, NA], F32, tag="pa")
                    nc.vector.tensor_scalar(out=pa[:], in0=ps_e[:1, :],
                                            scalar1=ob2_t[:1, :], scalar2=None,
                                            op0=OP.add)
                    en = sc.tile([1, NM], F32, tag="en")
                    nc.vector.tensor_reduce(
                        out=en[:], in_=pa[:].rearrange("p (m i) -> p m i", i=APM),
                        axis=AX.X, op=OP.add)
                    nc.sync.dma_start(out=out_d[:].unsqueeze(0), in_=en[:])

    nc.compile()
    return nc


def _prep_inputs(z, pos, ptr, emb, mlp_w1, mlp_b1, mlp_w2, mlp_b2,
                 lin1_w, lin2_w, lin2_b, lin_w, lin_b,
                 out_w1, out_b1, out_w2, out_b2):
    z = np.asarray(z)
    pos = np.ascontiguousarray(np.asarray(pos, dtype=np.float32))
    ptr = np.asarray(ptr)
    assert pos.shape == (N, 3)
    expect = np.arange(0, N + APM, APM)
    assert np.array_equal(ptr.astype(np.int64), expect), "non-uniform molecules unsupported"

    emb = np.asarray(emb, dtype=np.float32)
    w1 = np.asarray(mlp_w1, dtype=np.float32)
    w1rep = np.zeros((L, 128, FEAT), dtype=np.float32)
    for g in range(4):
        w1rep[:, 32 * g:32 * g + NG, :] = w1
    diag = np.zeros((128, APM), dtype=np.float32)
    for p in range(128):
        diag[p, p % APM] = 1.0
    offs = np.zeros((128, 1), dtype=np.float32)
    offvals = np.linspace(0.0, CUTOFF, NG).astype(np.float32)
    for p in range(128):
        if p % 32 < NG:
            offs[p, 0] = offvals[p % 32]

    shared = {
        "w1rep": w1rep,
        "w2": np.ascontiguousarray(mlp_w2, dtype=np.float32),
        "b1": np.ascontiguousarray(mlp_b1, dtype=np.float32),
        "b2": np.ascontiguousarray(mlp_b2, dtype=np.float32),
        "l1w": np.ascontiguousarray(lin1_w, dtype=np.float32),
        "l2w": np.ascontiguousarray(lin2_w, dtype=np.float32),
        "l2b": np.ascontiguousarray(lin2_b, dtype=np.float32),
        "lw": np.ascontiguousarray(lin_w, dtype=np.float32),
        "lb": np.ascontiguousarray(lin_b, dtype=np.float32),
        "ow1": np.ascontiguousarray(out_w1, dtype=np.float32),
        "ob1": np.ascontiguousarray(np.asarray(out_b1, dtype=np.float32)),
        "ow2": np.ascontiguousarray(out_w2, dtype=np.float32),
        "ob2": np.asarray(out_b2, dtype=np.float32).reshape(1),
        "diagc": diag,
        "offs": offs,
    }
    in_maps = []
    for c in range(NCORES):
        sl = slice(NA * c, NA * (c + 1))
        h0 = emb[np.asarray(z[sl], dtype=np.int64)].T
        m = dict(shared)
        m["pos"] = pos[sl].copy()
        m["h0"] = np.ascontiguousarray(h0, dtype=np.float32)
        in_maps.append(m)
    return in_maps


def kernel(**inputs) -> np.ndarray:
    from concourse.bass_utils import run_bass_kernel_spmd
    global _COMPILED
    if _COMPILED is None:
        _COMPILED = _build(1)
    nc = _COMPILED
    in_maps = _prep_inputs(**inputs)
    res = run_bass_kernel_spmd(nc, in_maps, list(range(NCORES)))
    out = np.concatenate([res.results[c]["energy"] for c in range(NCORES)])
    return out.astype(np.float32)


if __name__ == "__main__":
    _build(1)
    print("built ok")



# revision 2
# speedup vs baseline: 14.8055x; 14.8055x over previous
"""SchNet forward on 8 Trainium2 NeuronCores (Bass/Tile), data-parallel over molecules.

kernel(**inputs) takes FULL inputs (as produced by setup_inputs) and returns
the FULL [256] float32 per-molecule energies. Inside: shards 256 molecules
into 8 groups of 32 (1024 atoms each), runs an SPMD Bass kernel on cores 0-7,
gathers outputs.

Hardcoded shape: N=8192 atoms, 32 atoms/molecule, FEAT=100, NG=25, K=28, L=4,
CUTOFF=6.  Per core: 1024 atoms, all-pairs 32x32 block distances (E=32768
edge slots); top-28 selection done by rank counting; non-selected edges get
distance=CUTOFF so the cosine cutoff zeroes them exactly like the reference's
top_k + ccut weighting.
"""

import math
import numpy as np

N = 8192
APM = 32
FEAT = 100
NG = 25
K = 28
L = 4
CUTOFF = 6.0
NCORES = 8
NA = N // NCORES          # atoms per core = 1024
NM = NA // APM            # molecules per core = 32
E = NA * APM              # edge slots per core = 32768
EG = E // 4               # edges per partition-group = 8192
EC = 1024                 # edge chunk = one molecule's 32x32 pairs
H = FEAT // 2
NBLK = NA // 128          # 8 atom blocks per core

_COMPILED = None


def _build(repeats: int = 1):
    import concourse.bass as bass
    import concourse.mybir as mybir
    import concourse.tile as tile
    from concourse import bacc

    dt = mybir.dt
    F32 = dt.float32
    F32R = dt.float32r
    A = mybir.ActivationFunctionType
    OP = mybir.AluOpType
    AX = mybir.AxisListType
    LF = L * FEAT

    nc = bacc.Bacc()

    pos_d = nc.dram_tensor("pos", [NA, 3], F32, kind="ExternalInput")
    h0_d = nc.dram_tensor("h0", [FEAT, NA], F32, kind="ExternalInput")
    w1rep_d = nc.dram_tensor("w1rep", [L, 128, FEAT], F32, kind="ExternalInput")
    w2_d = nc.dram_tensor("w2", [L, FEAT, FEAT], F32, kind="ExternalInput")
    b1_d = nc.dram_tensor("b1", [L, FEAT], F32, kind="ExternalInput")
    b2_d = nc.dram_tensor("b2", [L, FEAT], F32, kind="ExternalInput")
    l1w_d = nc.dram_tensor("l1w", [L, FEAT, FEAT], F32, kind="ExternalInput")
    l2w_d = nc.dram_tensor("l2w", [L, FEAT, FEAT], F32, kind="ExternalInput")
    l2b_d = nc.dram_tensor("l2b", [L, FEAT], F32, kind="ExternalInput")
    lw_d = nc.dram_tensor("lw", [L, FEAT, FEAT], F32, kind="ExternalInput")
    lb_d = nc.dram_tensor("lb", [L, FEAT], F32, kind="ExternalInput")
    ow1_d = nc.dram_tensor("ow1", [FEAT, H], F32, kind="ExternalInput")
    ob1_d = nc.dram_tensor("ob1", [H], F32, kind="ExternalInput")
    ow2_d = nc.dram_tensor("ow2", [H, 1], F32, kind="ExternalInput")
    ob2_d = nc.dram_tensor("ob2", [1], F32, kind="ExternalInput")
    diag_d = nc.dram_tensor("diagc", [128, APM], F32, kind="ExternalInput")
    offs_d = nc.dram_tensor("offs", [128, 1], F32, kind="ExternalInput")

    out_d = nc.dram_tensor("energy", [NM], F32, kind="ExternalOutput")

    dtil_dram = nc.dram_tensor("dtil_lin", [E], F32)
    gam_dram = nc.dram_tensor("gam_lin", [E], F32R)

    def bap(a, off, dims):
        return bass.AP(tensor=a.tensor, offset=a.offset + off, ap=dims)

    with tile.TileContext(nc) as tc:
        import contextlib
        ctx = contextlib.ExitStack()
        with ctx:
            persist = ctx.enter_context(tc.tile_pool(name="persist", bufs=1))
            wpool = ctx.enter_context(tc.tile_pool(name="weights", bufs=1))
            psA = ctx.enter_context(tc.tile_pool(name="psA", bufs=2, space="PSUM"))
            psB = ctx.enter_context(tc.tile_pool(name="psB", bufs=2, space="PSUM"))

            # persistent tiles
            ea0 = persist.tile([128, EG], F32R, tag="ea0")   # groups 0(base0),1(base64)
            ea1 = persist.tile([128, EG], F32R, tag="ea1")   # groups 2(base0),3(base64)
            hA = persist.tile([FEAT, NA], F32, tag="hA")
            hB = persist.tile([FEAT, NA], F32, tag="hB")
            x1_t = persist.tile([FEAT, NA], F32, tag="x1")
            agg_t = persist.tile([FEAT, NA], F32, tag="agg")
            half_t = persist.tile([128, 1], F32, tag="half")
            nhalfpi_t = persist.tile([128, 1], F32, tag="nhalfpi")
            diag_t = persist.tile([128, APM], F32, tag="diag")
            offs_t = persist.tile([128, 1], F32, tag="offs")
            nc.vector.memset(half_t[:], 0.5)
            nc.vector.memset(nhalfpi_t[:], -math.pi / 2)
            nc.sync.dma_start(out=diag_t[:], in_=diag_d[:])
            nc.sync.dma_start(out=offs_t[:], in_=offs_d[:])

            # weights
            w1f = wpool.tile([128, LF], F32, tag="w1f")
            w1_t = wpool.tile([128, LF], F32R, tag="w1")
            w2f = wpool.tile([FEAT, LF], F32, tag="w2f")
            w2_t = wpool.tile([FEAT, LF], F32R, tag="w2")
            b2f = wpool.tile([128, LF], F32, tag="b2f")
            b2r_t = wpool.tile([128, LF], F32R, tag="b2r")   # row 64 holds b2 per layer
            l1w_t = wpool.tile([FEAT, LF], F32, tag="l1w")
            l2w_t = wpool.tile([FEAT, LF], F32, tag="l2w")
            lw_t = wpool.tile([FEAT, LF], F32, tag="lww")
            b1_t = wpool.tile([FEAT, L], F32, tag="b1")
            l2b_t = wpool.tile([FEAT, L], F32, tag="l2b")
            lb_t = wpool.tile([FEAT, L], F32, tag="lb")
            ow1_t = wpool.tile([FEAT, H], F32, tag="ow1")
            ob1_t = wpool.tile([H, 1], F32, tag="ob1")
            ow2_t = wpool.tile([H, 1], F32, tag="ow2")
            ob2_t = wpool.tile([1, 1], F32, tag="ob2")

            nc.sync.dma_start(out=w1f[:].rearrange("p (l f) -> p l f", f=FEAT),
                              in_=w1rep_d[:].transpose([1, 0, 2]))
            nc.vector.tensor_copy(w1_t[:], w1f[:])
            nc.sync.dma_start(out=w2f[:].rearrange("p (l f) -> p l f", f=FEAT),
                              in_=w2_d[:].transpose([1, 0, 2]))
            nc.vector.tensor_copy(w2_t[:], w2f[:])
            nc.vector.memset(b2f[:], 0.0)
            nc.sync.dma_start(
                out=b2f[64:65, :].rearrange("p (l f) -> p l f", f=FEAT),
                in_=b2_d[:].unsqueeze(0))
            nc.vector.tensor_copy(b2r_t[:], b2f[:])
            nc.sync.dma_start(out=l1w_t[:].rearrange("p (l f) -> p l f", f=FEAT),
                              in_=l1w_d[:].transpose([1, 0, 2]))
            nc.sync.dma_start(out=l2w_t[:].rearrange("p (l f) -> p l f", f=FEAT),
                              in_=l2w_d[:].transpose([1, 0, 2]))
            nc.sync.dma_start(out=lw_t[:].rearrange("p (l f) -> p l f", f=FEAT),
                              in_=lw_d[:].transpose([1, 0, 2]))
            nc.sync.dma_start(out=b1_t[:], in_=b1_d[:].transpose([1, 0]))
            nc.sync.dma_start(out=l2b_t[:], in_=l2b_d[:].transpose([1, 0]))
            nc.sync.dma_start(out=lb_t[:], in_=lb_d[:].transpose([1, 0]))
            nc.sync.dma_start(out=ow1_t[:], in_=ow1_d[:])
            nc.sync.dma_start(out=ob1_t[:], in_=ob1_d[:].unsqueeze(1))
            nc.sync.dma_start(out=ow2_t[:], in_=ow2_d[:])
            nc.sync.dma_start(out=ob2_t[:], in_=ob2_d[:].unsqueeze(1))

            with tc.For_i(0, repeats, 1):
                rep = 0
                nc.sync.dma_start(out=hA[:], in_=h0_d[:])
                pA = tc.tile_pool(name=f"bld{rep}", bufs=1)
                pAs = tc.tile_pool(name=f"scrA{rep}", bufs=2)
                with pA as bp, pAs as sc:
                    # ========== PHASE A: graph build ==========
                    EA_ = NBLK * APM
                    d2all = bp.tile([128, EA_], F32, tag="d2all")
                    for b in range(NBLK):
                        posP = sc.tile([128, 3], F32, tag="posP")
                        nc.sync.dma_start(out=posP[:], in_=pos_d[128 * b:128 * (b + 1), :])
                        posB = sc.tile([128, APM, 3], F32, tag="posB")
                        nc.sync.dma_start(
                            out=posB[:],
                            in_=bap(pos_d[:], 4 * b * APM * 3,
                                    [[APM * 3, 4], [0, APM], [3, APM], [1, 3]]))
                        dif = sc.tile([128, APM, 3], F32, tag="dif")
                        pP = posP[:]
                        nc.vector.tensor_tensor(
                            out=dif[:],
                            in0=bap(pP, 0, [pP.ap[0], [0, APM], [1, 3]]),
                            in1=posB[:], op=OP.subtract)
                        sq = sc.tile([128, APM, 3], F32, tag="sq")
                        nc.vector.tensor_tensor(out=sq[:], in0=dif[:], in1=dif[:],
                                                op=OP.mult)
                        nc.vector.tensor_reduce(out=d2all[:, APM * b:APM * (b + 1)],
                                                in_=sq[:], axis=AX.X, op=OP.add)
                    gtm = bp.tile([128, EA_], F32, tag="gtm")
                    nc.vector.tensor_scalar(out=gtm[:], in0=d2all[:], scalar1=36.0,
                                            scalar2=None, op0=OP.is_gt)
                    mask = bp.tile([128, EA_], F32, tag="mask")
                    for b in range(NBLK):
                        nc.vector.tensor_tensor(out=mask[:, APM * b:APM * (b + 1)],
                                                in0=gtm[:, APM * b:APM * (b + 1)],
                                                in1=diag_t[:], op=OP.max)
                    inv = bp.tile([128, EA_], F32, tag="inv")
                    nc.vector.tensor_scalar(out=inv[:], in0=mask[:], scalar1=-1.0,
                                            scalar2=1.0, op0=OP.mult, op1=OP.add)
                    d2m = bp.tile([128, EA_], F32, tag="d2m")
                    nc.vector.tensor_tensor(out=d2m[:], in0=d2all[:], in1=inv[:],
                                            op=OP.mult)
                    m36 = bp.tile([128, EA_], F32, tag="m36")
                    nc.vector.tensor_scalar(out=m36[:], in0=mask[:], scalar1=36.0,
                                            scalar2=None, op0=OP.mult)
                    nc.vector.tensor_tensor(out=d2m[:], in0=d2m[:], in1=m36[:],
                                            op=OP.add)

                    sel = bp.tile([128, EA_], F32, tag="sel")
                    for b in range(NBLK):
                        dd = d2m[:, APM * b:APM * (b + 1)]
                        lt = sc.tile([128, APM, APM], F32, tag="lt")
                        nc.vector.tensor_tensor(
                            out=lt[:],
                            in0=bap(dd, 0, [dd.ap[0], [0, APM], [1, APM]]),
                            in1=bap(dd, 0, [dd.ap[0], [1, APM], [0, APM]]),
                            op=OP.is_lt)
                        rk = sc.tile([128, APM], F32, tag="rk")
                        nc.vector.tensor_reduce(out=rk[:], in_=lt[:], axis=AX.X,
                                                op=OP.add)
                        nc.vector.tensor_scalar(out=sel[:, APM * b:APM * (b + 1)],
                                                in0=rk[:], scalar1=float(K) - 0.5,
                                                scalar2=None, op0=OP.is_lt)

                    s_t = bp.tile([128, EA_], F32, tag="s_t")
                    nc.scalar.activation(s_t[:], d2m[:], A.Sqrt)
                    for _ in range(2):
                        rc = sc.tile([128, EA_], F32, tag="rc")
                        nc.vector.reciprocal(rc[:], s_t[:])
                        tq = sc.tile([128, EA_], F32, tag="tq")
                        nc.vector.tensor_tensor(out=tq[:], in0=d2m[:], in1=rc[:],
                                                op=OP.mult)
                        nc.vector.tensor_tensor(out=s_t[:], in0=s_t[:], in1=tq[:],
                                                op=OP.add)
                        nc.vector.tensor_scalar(out=s_t[:], in0=s_t[:], scalar1=0.5,
                                                scalar2=None, op0=OP.mult)
                    dm6 = bp.tile([128, EA_], F32, tag="dm6")
                    nc.vector.tensor_scalar(out=dm6[:], in0=s_t[:], scalar1=-6.0,
                                            scalar2=None, op0=OP.add)
                    dtil = bp.tile([128, EA_], F32, tag="dtil")
                    nc.vector.tensor_tensor(out=dtil[:], in0=sel[:], in1=dm6[:],
                                            op=OP.mult)
                    nc.vector.tensor_scalar(out=dtil[:], in0=dtil[:], scalar1=6.0,
                                            scalar2=None, op0=OP.add)
                    sn = bp.tile([128, EA_], F32, tag="sn")
                    nc.scalar.activation(sn[:], dtil[:], A.Sin, bias=nhalfpi_t[:],
                                         scale=float(math.pi / 6.0))
                    nc.vector.tensor_scalar(out=sn[:], in0=sn[:], scalar1=-0.5,
                                            scalar2=0.5, op0=OP.mult, op1=OP.add)
                    ilt = bp.tile([128, EA_], F32, tag="ilt")
                    nc.vector.tensor_scalar(out=ilt[:], in0=d2m[:], scalar1=36.0,
                                            scalar2=None, op0=OP.is_lt)
                    nc.vector.tensor_tensor(out=ilt[:], in0=ilt[:], in1=sel[:],
                                            op=OP.mult)
                    gam = bp.tile([128, EA_], F32R, tag="gam")
                    nc.vector.tensor_tensor(out=gam[:], in0=sn[:], in1=ilt[:],
                                            op=OP.mult)

                    for b in range(NBLK):
                        nc.sync.dma_start(
                            out=bap(dtil_dram[:], 4096 * b, [[APM, 128], [1, APM]]),
                            in_=dtil[:, APM * b:APM * (b + 1)])
                        nc.sync.dma_start(
                            out=bap(gam_dram[:], 4096 * b, [[APM, 128], [1, APM]]),
                            in_=gam[:, APM * b:APM * (b + 1)])

                    # drep tiles: tile t holds groups {2t,2t+1} at bases {0,64}
                    for t_i, ea_tile in ((0, ea0), (1, ea1)):
                        drep = bp.tile([128, EG], F32, tag="drep")
                        for gg in range(2):
                            g = 2 * t_i + gg
                            dst = bap(drep[:], 0,
                                      [[drep[:].ap[0][0] * 64, 1],
                                       [drep[:].ap[0][0], 32], [1, EG]])
                            dst = bass.AP(tensor=drep[:].tensor,
                                          offset=drep[:].offset,
                                          ap=[[drep[:].ap[0][0], 32], [1, EG]]) \
                                if gg == 0 else \
                                bass.AP(tensor=drep[:].tensor,
                                        offset=drep[:].offset + 64 * drep[:].ap[0][0],
                                        ap=[[drep[:].ap[0][0], 32], [1, EG]])
                            nc.sync.dma_start(
                                out=dst,
                                in_=bap(dtil_dram[:], EG * g, [[0, 32], [1, EG]]))
                        for cj in range(EG // 2048):
                            ssl = slice(2048 * cj, 2048 * (cj + 1))
                            q = sc.tile([128, 2048], F32, tag="q")
                            nc.vector.tensor_scalar(out=q[:], in0=drep[:, ssl],
                                                    scalar1=offs_t[:], scalar2=None,
                                                    op0=OP.subtract)
                            nc.vector.tensor_tensor(out=q[:], in0=q[:], in1=q[:],
                                                    op=OP.mult)
                            nc.scalar.activation(ea_tile[:, ssl], q[:], A.Exp,
                                                 scale=-8.0)

                with tc.tile_pool(name=f"scrB{rep}", bufs=2) as sc:
                    # ========== PHASE B: interaction layers ==========
                    hcur, hnxt = hA, hB
                    for l in range(L):
                        lf = slice(FEAT * l, FEAT * (l + 1))
                        ps_n = psA.tile([FEAT, NA], F32, tag="psA")
                        for hh in range(2):
                            qs = slice(512 * hh, 512 * (hh + 1))
                            nc.tensor.matmul(ps_n[:, qs], l1w_t[:, lf], hcur[:, qs],
                                             start=True, stop=True)
                        nc.vector.tensor_copy(x1_t[:], ps_n[:])

                        for ci in range(E // EC):
                            g, cj = divmod(ci, EG // EC)
                            ea_tile = ea0 if g < 2 else ea1
                            base = 64 * (g % 2)
                            ps1 = psA.tile([FEAT, EC], F32, tag="psA")
                            for q2 in range(EC // 512):
                                qs = slice(512 * q2, 512 * (q2 + 1))
                                nc.tensor.matmul(
                                    ps1[:, qs],
                                    w1_t[base:base + NG, lf],
                                    ea_tile[base:base + NG,
                                            EC * cj + 512 * q2:EC * cj + 512 * (q2 + 1)],
                                    start=True, stop=True)
                            ue = sc.tile([FEAT, EC], F32, tag="ue")
                            nc.scalar.activation(ue[:], ps1[:], A.Exp,
                                                 bias=b1_t[:, l:l + 1])
                            u = sc.tile([FEAT, EC], F32, tag="u")
                            nc.scalar.activation(u[:], ue[:], A.Ln,
                                                 bias=half_t[:FEAT], scale=0.5)
                            gr = sc.tile([128, EC], F32R, tag="gr")
                            nc.sync.dma_start(
                                out=gr[:],
                                in_=bap(gam_dram[:], EG * g + EC * cj,
                                        [[0, 128], [1, EC]]))
                            up = sc.tile([FEAT, EC], F32R, tag="up")
                            nc.vector.tensor_tensor(out=up[:], in0=u[:],
                                                    in1=gr[:FEAT, :], op=OP.mult)
                            ps2 = psB.tile([FEAT, EC], F32, tag="psB")
                            for q2 in range(EC // 512):
                                qs = slice(512 * q2, 512 * (q2 + 1))
                                nc.tensor.matmul(ps2[:, qs], w2_t[:, lf], up[:, qs],
                                                 start=True, stop=False)
                                nc.tensor.matmul(ps2[:, qs], b2r_t[64:65, lf],
                                                 gr[64:65, qs], start=False, stop=True)
                            a0 = 256 * g + 32 * cj   # first atom of this molecule
                            x1b = x1_t[:]
                            msg = sc.tile([FEAT, EC], F32, tag="msg")
                            nc.vector.tensor_tensor(
                                out=msg[:], in0=ps2[:],
                                in1=bap(x1b, a0, [x1b.ap[0], [0, APM], [1, APM]]),
                                op=OP.mult)
                            nc.vector.tensor_reduce(
                                out=agg_t[:, a0:a0 + APM],
                                in_=msg[:].rearrange("p (a j) -> p a j", j=APM),
                                axis=AX.X, op=OP.add)

                        ps_v = psA.tile([FEAT, NA], F32, tag="psA")
                        for hh in range(2):
                            qs = slice(512 * hh, 512 * (hh + 1))
                            nc.tensor.matmul(ps_v[:, qs], l2w_t[:, lf], agg_t[:, qs],
                                             start=True, stop=True)
                        spe = sc.tile([FEAT, NA], F32, tag="ue")
                        nc.scalar.activation(spe[:], ps_v[:], A.Exp,
                                             bias=l2b_t[:, l:l + 1])
                        spl = sc.tile([FEAT, NA], F32, tag="u")
                        nc.scalar.activation(spl[:], spe[:], A.Ln,
                                             bias=half_t[:FEAT], scale=0.5)
                        ps_w = psB.tile([FEAT, NA], F32, tag="psB")
                        for hh in range(2):
                            qs = slice(512 * hh, 512 * (hh + 1))
                            nc.tensor.matmul(ps_w[:, qs], lw_t[:, lf], spl[:, qs],
                                             start=True, stop=True)
                        nc.vector.scalar_tensor_tensor(
                            out=hnxt[:], in0=ps_w[:], scalar=lb_t[:, l:l + 1],
                            in1=hcur[:], op0=OP.add, op1=OP.add)
                        hcur, hnxt = hnxt, hcur

                    # ========== PHASE C: readout ==========
                    ps_r = psA.tile([FEAT, NA], F32, tag="psA")
                    for hh in range(2):
                        qs = slice(512 * hh, 512 * (hh + 1))
                        nc.tensor.matmul(ps_r[:H, qs], ow1_t[:], hcur[:, qs],
                                         start=True, stop=True)
                    re = sc.tile([H, NA], F32, tag="ue")
                    nc.scalar.activation(re[:], ps_r[:H, :], A.Exp, bias=ob1_t[:])
                    rl = sc.tile([H, NA], F32, tag="u")
                    nc.scalar.activation(rl[:], re[:], A.Ln, bias=half_t[:H],
                                         scale=0.5)
                    ps_e = psB.tile([FEAT, NA], F32, tag="psB")
                    for hh in range(2):
                        qs = slice(512 * hh, 512 * (hh + 1))
                        nc.tensor.matmul(ps_e[:1, qs], ow2_t[:], rl[:, qs],
                                         start=True, stop=True)
                    pa = sc.tile([1, NA], F32, tag="pa")
                    nc.vector.tensor_scalar(out=pa[:], in0=ps_e[:1, :],
                                            scalar1=ob2_t[:1, :], scalar2=None,
                                            op0=OP.add)
                    en = sc.tile([1, NM], F32, tag="en")
                    nc.vector.tensor_reduce(
                        out=en[:], in_=pa[:].rearrange("p (m i) -> p m i", i=APM),
                        axis=AX.X, op=OP.add)
                    nc.sync.dma_start(out=out_d[:].unsqueeze(0), in_=en[:])

    nc.compile()
    return nc


def _prep_inputs(z, pos, ptr, emb, mlp_w1, mlp_b1, mlp_w2, mlp_b2,
                 lin1_w, lin2_w, lin2_b, lin_w, lin_b,
                 out_w1, out_b1, out_w2, out_b2):
    z = np.asarray(z)
    pos = np.ascontiguousarray(np.asarray(pos, dtype=np.float32))
    ptr = np.asarray(ptr)
    assert pos.shape == (N, 3)
    expect = np.arange(0, N + APM, APM)
    assert np.array_equal(ptr.astype(np.int64), expect), "non-uniform molecules unsupported"

    emb = np.asarray(emb, dtype=np.float32)
    w1 = np.asarray(mlp_w1, dtype=np.float32)
    w1rep = np.zeros((L, 128, FEAT), dtype=np.float32)
    for g in range(4):
        w1rep[:, 32 * g:32 * g + NG, :] = w1
    diag = np.zeros((128, APM), dtype=np.float32)
    for p in range(128):
        diag[p, p % APM] = 1.0
    offs = np.zeros((128, 1), dtype=np.float32)
    offvals = np.linspace(0.0, CUTOFF, NG).astype(np.float32)
    for p in range(128):
        if p % 32 < NG:
            offs[p, 0] = offvals[p % 32]

    shared = {
        "w1rep": w1rep,
        "w2": np.ascontiguousarray(mlp_w2, dtype=np.float32),
        "b1": np.ascontiguousarray(mlp_b1, dtype=np.float32),
        "b2": np.ascontiguousarray(mlp_b2, dtype=np.float32),
        "l1w": np.ascontiguousarray(lin1_w, dtype=np.float32),
        "l2w": np.ascontiguousarray(lin2_w, dtype=np.float32),
        "l2b": np.ascontiguousarray(lin2_b, dtype=np.float32),
        "lw": np.ascontiguousarray(lin_w, dtype=np.float32),
        "lb": np.ascontiguousarray(lin_b, dtype=np.float32),
        "ow1": np.ascontiguousarray(out_w1, dtype=np.float32),
        "ob1": np.ascontiguousarray(np.asarray(out_b1, dtype=np.float32)),
        "ow2": np.ascontiguousarray(out_w2, dtype=np.float32),
        "ob2": np.asarray(out_b2, dtype=np.float32).reshape(1),
        "diagc": diag,
        "offs": offs,
    }
    in_maps = []
    for c in range(NCORES):
        sl = slice(NA * c, NA * (c + 1))
        h0 = emb[np.asarray(z[sl], dtype=np.int64)].T
        m = dict(shared)
        m["pos"] = pos[sl].copy()
        m["h0"] = np.ascontiguousarray(h0, dtype=np.float32)
        in_maps.append(m)
    return in_maps


def kernel(**inputs) -> np.ndarray:
    from concourse.bass_utils import run_bass_kernel_spmd
    global _COMPILED
    if _COMPILED is None:
        _COMPILED = _build(1)
    nc = _COMPILED
    in_maps = _prep_inputs(**inputs)
    res = run_bass_kernel_spmd(nc, in_maps, list(range(NCORES)))
    out = np.concatenate([res.results[c]["energy"] for c in range(NCORES)])
    return out.astype(np.float32)


if __name__ == "__main__":
    _build(1)
    print("built ok")



# revision 51
# speedup vs baseline: 297.6694x; 20.1053x over previous
"""SchNet forward on 8 Trainium2 NeuronCores (Bass/Tile), data-parallel over molecules.

kernel(**inputs) takes FULL inputs (as produced by setup_inputs) and returns
the FULL [256] float32 per-molecule energies. Inside: shards 256 molecules
into 8 groups of 32 (1024 atoms each), runs an SPMD Bass kernel on cores 0-7,
gathers outputs.

Hardcoded shape: N=8192 atoms, 32 atoms/molecule, FEAT=100, NG=25, K=28, L=4,
CUTOFF=6.  Per core: 1024 atoms, all-pairs 32x32 block distances (E=32768
edge slots); top-28 selection via rank counting; non-selected edges get
distance=CUTOFF so the cosine cutoff zeroes them exactly like the reference's
top_k + ccut weighting.

v3 design notes:
 - repeats is a hardware For_i loop (NEFF size independent of repeat count).
 - atoms mapped p-major in phase A: atom a = 8p + k on partition p; molecule
   m = p//4; in-molecule index i = 8*(p%4) + k. Column c = k*32 + j, so a
   partition-major [128,256] -> DRAM write is exactly edge order e = a*32+j.
 - The edge filter W(d) = ssp(gauss(d) @ w1 + b1) @ w2 + b2 is a smooth scalar
   function of edge distance only, so it is refit host-side (ridge lstsq) onto
   a richer 51-Gaussian basis; the per-edge MLP collapses into one matmul with
   the fitted coefficients.  The cutoff weight gamma is folded into the basis
   values (and the fitted constant term rides on the Gaussians), so the whole
   per-edge filter application is: W*gamma = alpha^T @ (gauss51(d) * gamma).
 - basis values bf16, built once per rep; per-edge cutoff gamma broadcast via
   ones-vector matmuls (no 128-descriptor broadcast DMAs in inner loops).
"""

import math
import numpy as np

N = 8192
APM = 32
FEAT = 100
NG = 25
K = 28
L = 4
CUTOFF = 6.0
NCORES = 8
NA = N // NCORES          # atoms per core = 1024
NM = NA // APM            # molecules per core = 32
E = NA * APM              # edge slots per core = 32768
EG = E // 2               # edges per partition-group = 16384
EC = APM * APM            # edge chunk = one molecule's 32x32 pairs = 1024
H = FEAT // 2
NB = 51                   # fitted gaussian basis size
GCOEFF = -8.0             # gaussian exponent coefficient
RIDGE = 1e-4              # ridge strength for the filter refit
LOG2 = float(np.log(2.0))


_COMPILED = None


def _build(repeats: int = 1, hw_loop: bool = True, debug: bool = False,
           accum_out: bool = False):
    import contextlib
    import concourse.bass as bass
    import concourse.mybir as mybir
    import concourse.tile as tile
    from concourse import bacc

    dt = mybir.dt
    F32 = dt.float32
    BF16 = dt.bfloat16
    A = mybir.ActivationFunctionType
    OP = mybir.AluOpType
    AX = mybir.AxisListType
    LF = L * FEAT

    nc = bacc.Bacc()

    pos_d = nc.dram_tensor("pos", [NA, 3], F32, kind="ExternalInput")
    h0_d = nc.dram_tensor("h0", [FEAT, NA], BF16, kind="ExternalInput")
    alph_d = nc.dram_tensor("alph", [L, 128, FEAT], BF16, kind="ExternalInput")
    l1w_d = nc.dram_tensor("l1w", [L, FEAT, FEAT], F32, kind="ExternalInput")
    l2w_d = nc.dram_tensor("l2w", [L, FEAT, FEAT], F32, kind="ExternalInput")
    l2b_d = nc.dram_tensor("l2b", [FEAT, L], F32, kind="ExternalInput")
    lw_d = nc.dram_tensor("lw", [L, FEAT, FEAT], F32, kind="ExternalInput")
    lb_d = nc.dram_tensor("lb", [FEAT, L], F32, kind="ExternalInput")
    ow1_d = nc.dram_tensor("ow1", [FEAT, H], F32, kind="ExternalInput")
    ob1_d = nc.dram_tensor("ob1", [H], F32, kind="ExternalInput")
    ow2_d = nc.dram_tensor("ow2", [H, 1], F32, kind="ExternalInput")
    ob2_d = nc.dram_tensor("ob2", [1], F32, kind="ExternalInput")
    diag_d = nc.dram_tensor("diagbig", [128, 8 * APM], BF16, kind="ExternalInput")
    offs_d = nc.dram_tensor("offs", [128, 1], F32, kind="ExternalInput")
    cmat_d = nc.dram_tensor("cmat", [2, 128], F32, kind="ExternalInput")

    out_d = nc.dram_tensor("energy", [NM], F32, kind="ExternalOutput")

    dtil_dram = nc.dram_tensor("dtil_lin", [E], F32)
    qrow_dram = nc.dram_tensor("qrow_lin", [E], F32)
    if debug:
        dbg_ea = nc.dram_tensor("dbg_ea", [128, EG], F32, kind="ExternalOutput")
        dbg_x1 = nc.dram_tensor("dbg_x1", [FEAT, NA], F32, kind="ExternalOutput")
        dbg_agg = nc.dram_tensor("dbg_agg", [FEAT, NA], F32, kind="ExternalOutput")
        dbg_ps2 = nc.dram_tensor("dbg_ps2", [FEAT, EC], F32, kind="ExternalOutput")
        dbg_msg = nc.dram_tensor("dbg_msg", [FEAT, 2 * EC], F32,
                                 kind="ExternalOutput")
        dbg_al = nc.dram_tensor("dbg_al", [128, L * FEAT], F32,
                                kind="ExternalOutput")

    def bap(a, off, dims):
        return bass.AP(tensor=a.tensor, offset=a.offset + off, ap=dims)

    with tile.TileContext(nc) as tc:
        ctx = contextlib.ExitStack()
        with ctx:
            persist = ctx.enter_context(tc.tile_pool(name="persist", bufs=1))
            wpool = ctx.enter_context(tc.tile_pool(name="weights", bufs=1))
            psA = ctx.enter_context(tc.tile_pool(name="psA", bufs=2, space="PSUM"))
            psB = ctx.enter_context(tc.tile_pool(name="psB", bufs=2, space="PSUM"))

            # ---- persistent tiles
            ea_t = persist.tile([128, EG], BF16, tag="ea")  # 2 groups @ bases 0/64
            hA = persist.tile([FEAT, NA], F32, tag="hA")
            hB = persist.tile([FEAT, NA], F32, tag="hB")
            x1_t = persist.tile([FEAT, NA], F32, tag="x1")
            agg_t = persist.tile([FEAT, NA], F32, tag="agg")
            h0b_t = persist.tile([FEAT, NA], BF16, tag="h0b")
            nhalfpi_t = persist.tile([128, 1], F32, tag="nhalfpi")
            half_t = persist.tile([128, 1], F32, tag="half")
            diag16_t = persist.tile([128, 8 * APM], BF16, tag="diag16")
            diag_t = persist.tile([128, 8 * APM], F32, tag="diag")
            offs_t = persist.tile([128, 1], F32, tag="offs")
            cmat_t = persist.tile([2, 128], F32, tag="cmat")
            tiny_t = persist.tile([128, 1], F32, tag="tiny")
            nc.vector.memset(nhalfpi_t[:], -math.pi / 2)
            nc.vector.memset(half_t[:], 0.5)
            nc.vector.memset(tiny_t[:], 1e-30)
            nc.sync.dma_start(out=diag16_t[:], in_=diag_d[:])
            nc.sync.dma_start(out=offs_t[:], in_=offs_d[:])
            nc.sync.dma_start(out=cmat_t[:], in_=cmat_d[:])
            nc.scalar.activation(diag_t[:], diag16_t[:], A.Copy)

            # ---- weights
            alph_t = wpool.tile([128, LF], BF16, tag="alph")
            l1w_t = wpool.tile([FEAT, LF], F32, tag="l1w")
            l2w_t = wpool.tile([FEAT, LF], F32, tag="l2w")
            lw_t = wpool.tile([FEAT, LF], F32, tag="lww")
            l2b_t = wpool.tile([FEAT, L], F32, tag="l2b")
            lb_t = wpool.tile([FEAT, L], F32, tag="lb")
            ow1_t = wpool.tile([FEAT, H], F32, tag="ow1")
            ob1_t = wpool.tile([H, 1], F32, tag="ob1")
            ow2_t = wpool.tile([H, 1], F32, tag="ow2")
            ob2_t = wpool.tile([1, 1], F32, tag="ob2")

            nc.sync.dma_start(out=alph_t[:].rearrange("p (l f) -> p l f", f=FEAT),
                              in_=alph_d[:].transpose([1, 0, 2]))
            nc.sync.dma_start(out=l1w_t[:].rearrange("p (l f) -> p l f", f=FEAT),
                              in_=l1w_d[:].transpose([1, 0, 2]))
            nc.sync.dma_start(out=l2w_t[:].rearrange("p (l f) -> p l f", f=FEAT),
                              in_=l2w_d[:].transpose([1, 0, 2]))
            nc.sync.dma_start(out=lw_t[:].rearrange("p (l f) -> p l f", f=FEAT),
                              in_=lw_d[:].transpose([1, 0, 2]))
            nc.sync.dma_start(out=l2b_t[:], in_=l2b_d[:])
            nc.sync.dma_start(out=lb_t[:], in_=lb_d[:])
            nc.sync.dma_start(out=ow1_t[:], in_=ow1_d[:])
            nc.sync.dma_start(out=ob1_t[:], in_=ob1_d[:].unsqueeze(1))
            nc.sync.dma_start(out=ow2_t[:], in_=ow2_d[:])
            nc.sync.dma_start(out=ob2_t[:], in_=ob2_d[:].unsqueeze(1))

            if accum_out:
                acc_en = persist.tile([1, NM], F32, tag="acc_en")
                nc.vector.memset(acc_en[:], 0.0)
            loop_cm = (tc.For_i(0, repeats, 1) if hw_loop
                       else contextlib.nullcontext())
            with loop_cm:
                nc.sync.dma_start(out=h0b_t[:], in_=h0_d[:])
                nc.scalar.activation(hA[:], h0b_t[:], A.Copy)

                with tc.tile_pool(name="bld", bufs=1) as bp, \
                     tc.tile_pool(name="scrA", bufs=2) as sc:
                    # ========== PHASE A: graph build ==========
                    # atom a = 8p + k; molecule p//4; col c = 32k + j
                    posC = bp.tile([128, 8, 3], F32, tag="posC")
                    nc.sync.dma_start(
                        out=posC[:],
                        in_=bap(pos_d[:], 0, [[24, 128], [3, 8], [1, 3]]))
                    posB = bp.tile([128, APM, 3], F32, tag="posB")
                    nc.sync.dma_start(
                        out=posB[:],
                        in_=bap(pos_d[:], 0, [[96, 32], [0, 4], [1, 96]]))
                    dif = bp.tile([128, 8, APM, 3], F32, tag="dif")
                    pC = posC[:]
                    pB = posB[:]
                    nc.vector.tensor_tensor(
                        out=dif[:],
                        in0=bap(pC, 0, [pC.ap[0], [3, 8], [0, APM], [1, 3]]),
                        in1=bap(pB, 0, [pB.ap[0], [0, 8], [3, APM], [1, 3]]),
                        op=OP.subtract)
                    sq = bp.tile([128, 8, APM, 3], F32, tag="sq")
                    nc.scalar.activation(sq[:], dif[:], A.Square)
                    d2r = bp.tile([128, 8 * APM], F32, tag="d2r")
                    nc.vector.tensor_reduce(
                        out=d2r[:].rearrange("p (k j) -> p k j", j=APM),
                        in_=sq[:], axis=AX.X, op=OP.add)
                    # mask: diagonal + cutoff -> exactly 36 (diag tile holds 36)
                    d2m = bp.tile([128, 8 * APM], F32, tag="d2m")
                    nc.vector.scalar_tensor_tensor(
                        out=d2m[:], in0=d2r[:], scalar=36.0, in1=diag_t[:],
                        op0=OP.min, op1=OP.max)
                    # rank-based top-K selection (strict-less rank within molecule)
                    dd = d2m[:]
                    lt = bp.tile([128, 8, APM, APM], BF16, tag="lt")
                    nc.vector.tensor_tensor(
                        out=lt[:],
                        in0=bap(dd, 0, [dd.ap[0], [APM, 8], [0, APM], [1, APM]]),
                        in1=bap(dd, 0, [dd.ap[0], [APM, 8], [1, APM], [0, APM]]),
                        op=OP.is_lt)
                    rk = bp.tile([128, 8 * APM], F32, tag="rk")
                    nc.vector.tensor_reduce(
                        out=rk[:].rearrange("p (k j) -> p k j", j=APM),
                        in_=lt[:], axis=AX.X, op=OP.add)
                    sel = bp.tile([128, 8 * APM], F32, tag="sel")
                    nc.vector.tensor_scalar(out=sel[:], in0=rk[:],
                                            scalar1=float(K) - 0.5,
                                            scalar2=None, op0=OP.is_lt)
                    # dist = sqrt(d2m) with one Newton refinement
                    s_t = bp.tile([128, 8 * APM], F32, tag="s_t")
                    nc.scalar.activation(s_t[:], d2m[:], A.Sqrt)
                    rc = sc.tile([128, 8 * APM], F32, tag="rc")
                    nc.vector.reciprocal(rc[:], s_t[:])
                    tq = sc.tile([128, 8 * APM], F32, tag="tq")
                    nc.vector.tensor_tensor(out=tq[:], in0=d2m[:], in1=rc[:],
                                            op=OP.mult)
                    nc.vector.tensor_tensor(out=s_t[:], in0=s_t[:], in1=tq[:],
                                            op=OP.add)
                    # dtil = sel ? 0.5*s : 6
                    dtil = bp.tile([128, 8 * APM], F32, tag="dtil")
                    nc.vector.tensor_scalar(out=dtil[:], in0=s_t[:], scalar1=0.5,
                                            scalar2=-6.0, op0=OP.mult, op1=OP.add)
                    nc.vector.tensor_tensor(out=dtil[:], in0=dtil[:], in1=sel[:],
                                            op=OP.mult)
                    nc.vector.tensor_scalar(out=dtil[:], in0=dtil[:], scalar1=6.0,
                                            scalar2=None, op0=OP.add)
                    # gamma = 0.5*(cos(pi*d/6)+1) * sel * (d2m<36)
                    sn = bp.tile([128, 8 * APM], F32, tag="sn")
                    nc.scalar.activation(sn[:], dtil[:], A.Sin, bias=nhalfpi_t[:],
                                         scale=float(math.pi / 6.0))
                    nc.vector.tensor_scalar(out=sn[:], in0=sn[:], scalar1=-0.5,
                                            scalar2=0.5, op0=OP.mult, op1=OP.add)
                    ilt = bp.tile([128, 8 * APM], F32, tag="ilt")
                    nc.vector.scalar_tensor_tensor(
                        out=ilt[:], in0=d2m[:], scalar=36.0, in1=sel[:],
                        op0=OP.is_lt, op1=OP.mult)
                    gamf = bp.tile([128, 8 * APM], F32, tag="gamf")
                    nc.vector.tensor_tensor(out=gamf[:], in0=sn[:], in1=ilt[:],
                                            op=OP.mult)
                    # q = -8*d^2 + ln(gamma): basis exp(16c*d - 8c^2 + q)
                    gamln = bp.tile([128, 8 * APM], F32, tag="gamln")
                    nc.scalar.activation(gamln[:], gamf[:], A.Ln, bias=tiny_t[:])
                    t2 = sc.tile([128, 8 * APM], F32, tag="t2")
                    nc.scalar.activation(t2[:], dtil[:], A.Square)
                    qrow_t = bp.tile([128, 8 * APM], F32, tag="qrow")
                    nc.vector.scalar_tensor_tensor(
                        out=qrow_t[:], in0=t2[:], scalar=float(GCOEFF),
                        in1=gamln[:], op0=OP.mult, op1=OP.add)
                    # p-major col-contiguous writes == edge order e = a*32+j.
                    # Split into 16 slices so they spread across DMA queues and
                    # each row-read below can start as soon as its slice lands.
                    for ws in range(16):
                        psl = slice(8 * ws, 8 * (ws + 1))
                        nc.sync.dma_start(
                            out=bap(dtil_dram[:], 2048 * ws,
                                    [[8 * APM, 8], [1, 8 * APM]]),
                            in_=dtil[psl, :])
                        nc.sync.dma_start(
                            out=bap(qrow_dram[:], 2048 * ws,
                                    [[8 * APM, 8], [1, 8 * APM]]),
                            in_=qrow_t[psl, :])

                    # basis build: ea[b, e] = exp(-8*(d_e - cent_b)^2) * gamma_e
                    #            = Exp(16*c_b*d_e + q_e - 8*c_b^2)
                    # group g2 on partitions 64*g2 .. 64*g2+50
                    for s32 in range(16):
                        row2 = sc.tile([2, 2048], F32, tag="row2")
                        nc.sync.dma_start(
                            out=row2[0:1, :],
                            in_=dtil_dram[2048 * s32:2048 * (s32 + 1)].unsqueeze(0))
                        nc.sync.dma_start(
                            out=row2[1:2, :],
                            in_=qrow_dram[2048 * s32:2048 * (s32 + 1)].unsqueeze(0))
                        g2 = s32 // 8
                        rows = slice(64 * g2, 64 * g2 + NB)
                        for q in range(2):
                            psQ = psA.tile([128, EC], F32, tag="psA")
                            for h2 in range(2):
                                cs = 1024 * q + 512 * h2
                                nc.tensor.matmul(
                                    psQ[:, 512 * h2:512 * (h2 + 1)],
                                    cmat_t[:2, :],
                                    row2[:2, cs:cs + 512],
                                    start=True, stop=True)
                            cb = 2048 * (s32 % 8) + 1024 * q
                            nc.scalar.activation(ea_t[rows, cb:cb + EC],
                                                 psQ[rows, :], A.Exp,
                                                 bias=offs_t[rows, :])

                with tc.tile_pool(name="scrB", bufs=2) as sc:
                    # ========== PHASE B: interaction layers ==========
                    if debug:
                        dal = sc.tile([128, L * FEAT], F32, tag="dal")
                        nc.vector.tensor_copy(dal[:], alph_t[:])
                        nc.sync.dma_start(out=dbg_al[:], in_=dal[:])
                        for dc in range(8):
                            ds_ = slice(2048 * dc, 2048 * (dc + 1))
                            for g2 in range(2):
                                rows = slice(64 * g2, 64 * g2 + NB)
                                eaf = sc.tile([128, 2048], F32, tag="eaf")
                                nc.scalar.activation(eaf[rows, :],
                                                     ea_t[rows, ds_], A.Copy)
                                nc.sync.dma_start(out=dbg_ea[rows, ds_],
                                                  in_=eaf[rows, :])
                    hcur, hnxt = hA, hB
                    for l in range(L):
                        lf = slice(FEAT * l, FEAT * (l + 1))
                        ps_n = psA.tile([128, NA], F32, tag="psA")
                        for hh in range(2):
                            qs = slice(512 * hh, 512 * (hh + 1))
                            nc.tensor.matmul(ps_n[:FEAT, qs], l1w_t[:, lf],
                                             hcur[:, qs], start=True, stop=True)
                        nc.scalar.activation(x1_t[:], ps_n[:FEAT, :], A.Copy)

                        for mp in range(NM // 2):
                            msg = sc.tile([FEAT, 2 * EC], BF16, tag="msg")
                            for hf in range(2):
                                m = 2 * mp + hf
                                base = 64 * (m // 16)
                                cb = EC * (m % 16)
                                a0 = APM * m
                                ps2 = psB.tile([128, EC], F32, tag="psB")
                                for q2 in range(2):
                                    qs = slice(512 * q2, 512 * (q2 + 1))
                                    nc.tensor.matmul(
                                        ps2[:FEAT, qs],
                                        alph_t[base:base + NB, lf],
                                        ea_t[base:base + NB,
                                             cb + 512 * q2:cb + 512 * (q2 + 1)],
                                        start=True, stop=True)
                                x1b = x1_t[:]
                                if debug and l == 0 and mp == 0 and hf == 0:
                                    dps = sc.tile([FEAT, EC], F32, tag="dps")
                                    nc.vector.tensor_copy(dps[:], ps2[:FEAT, :])
                                    nc.sync.dma_start(out=dbg_ps2[:], in_=dps[:])
                                nc.vector.tensor_tensor(
                                    out=msg[:, EC * hf:EC * (hf + 1)],
                                    in0=ps2[:FEAT, :],
                                    in1=bap(x1b, a0,
                                            [x1b.ap[0], [0, APM], [1, APM]]),
                                    op=OP.mult)
                            if debug and l == 0 and mp == 0:
                                dmsg = sc.tile([FEAT, 2 * EC], F32, tag="dmsg")
                                nc.vector.tensor_copy(dmsg[:], msg[:])
                                nc.sync.dma_start(out=dbg_msg[:], in_=dmsg[:])
                            a0p = 2 * APM * mp
                            nc.vector.tensor_reduce(
                                out=agg_t[:, a0p:a0p + 2 * APM],
                                in_=msg[:].rearrange("p (a j) -> p a j", j=APM),
                                axis=AX.X, op=OP.add)

                        if debug and l == 0:
                            nc.sync.dma_start(out=dbg_x1[:], in_=x1_t[:])
                            nc.sync.dma_start(out=dbg_agg[:], in_=agg_t[:])
                        ps_v = psA.tile([128, NA], F32, tag="psA")
                        for hh in range(2):
                            qs = slice(512 * hh, 512 * (hh + 1))
                            nc.tensor.matmul(ps_v[:FEAT, qs], l2w_t[:, lf],
                                             agg_t[:, qs], start=True, stop=True)
                        spe = sc.tile([FEAT, NA], F32, tag="spe")
                        nc.scalar.activation(spe[:], ps_v[:FEAT, :], A.Exp,
                                             bias=l2b_t[:, l:l + 1])
                        spl = sc.tile([FEAT, NA], F32, tag="spl")
                        nc.scalar.activation(spl[:], spe[:], A.Ln,
                                             bias=half_t[:FEAT], scale=0.5)
                        ps_w = psB.tile([128, NA], F32, tag="psB")
                        for hh in range(2):
                            qs = slice(512 * hh, 512 * (hh + 1))
                            nc.tensor.matmul(ps_w[:FEAT, qs], lw_t[:, lf],
                                             spl[:, qs], start=True, stop=True)
                        nc.vector.scalar_tensor_tensor(
                            out=hnxt[:], in0=ps_w[:FEAT, :],
                            scalar=lb_t[:, l:l + 1],
                            in1=hcur[:], op0=OP.add, op1=OP.add)
                        hcur, hnxt = hnxt, hcur

                    # ========== PHASE C: readout ==========
                    ps_r = psA.tile([128, NA], F32, tag="psA")
                    for hh in range(2):
                        qs = slice(512 * hh, 512 * (hh + 1))
                        nc.tensor.matmul(ps_r[:H, qs], ow1_t[:], hcur[:, qs],
                                         start=True, stop=True)
                    ree = sc.tile([H, NA], F32, tag="spe")
                    nc.scalar.activation(ree[:], ps_r[:H, :], A.Exp,
                                         bias=ob1_t[:])
                    re = sc.tile([H, NA], F32, tag="spl")
                    nc.scalar.activation(re[:], ree[:], A.Ln,
                                         bias=half_t[:H], scale=0.5)
                    ps_e = psB.tile([128, NA], F32, tag="psB")
                    for hh in range(2):
                        qs = slice(512 * hh, 512 * (hh + 1))
                        nc.tensor.matmul(ps_e[:1, qs], ow2_t[:], re[:, qs],
                                         start=True, stop=True)
                    pa = sc.tile([1, NA], F32, tag="pa")
                    nc.vector.tensor_scalar(out=pa[:], in0=ps_e[:1, :],
                                            scalar1=ob2_t[:1, :], scalar2=None,
                                            op0=OP.add)
                    en = sc.tile([1, NM], F32, tag="en")
                    nc.vector.tensor_reduce(
                        out=en[:], in_=pa[:].rearrange("p (m i) -> p m i", i=APM),
                        axis=AX.X, op=OP.add)
                    if accum_out:
                        nc.vector.tensor_tensor(out=acc_en[:], in0=acc_en[:],
                                                in1=en[:], op=OP.add)
                        nc.sync.dma_start(out=out_d[:].unsqueeze(0),
                                          in_=acc_en[:])
                    else:
                        nc.sync.dma_start(out=out_d[:].unsqueeze(0), in_=en[:])

    nc.compile()
    return nc


def _prep_inputs(z, pos, ptr, emb, mlp_w1, mlp_b1, mlp_w2, mlp_b2,
                 lin1_w, lin2_w, lin2_b, lin_w, lin_b,
                 out_w1, out_b1, out_w2, out_b2):
    import ml_dtypes
    BF = ml_dtypes.bfloat16

    z = np.asarray(z)
    pos = np.ascontiguousarray(np.asarray(pos, dtype=np.float32))
    ptr = np.asarray(ptr)
    assert pos.shape == (N, 3)
    expect = np.arange(0, N + APM, APM)
    assert np.array_equal(ptr.astype(np.int64), expect), "non-uniform molecules unsupported"

    emb = np.asarray(emb, dtype=np.float32)
    w1 = np.asarray(mlp_w1, dtype=np.float64)        # [L, NG, FEAT]
    b1 = np.asarray(mlp_b1, dtype=np.float64)
    w2 = np.asarray(mlp_w2, dtype=np.float64)
    b2 = np.asarray(mlp_b2, dtype=np.float64)

    # refit the edge filter (a smooth function of distance) onto NB gaussians
    cent = np.linspace(0.0, CUTOFF, NB)
    grid = np.linspace(0.0, CUTOFF, 1201)
    Phi = np.exp(GCOEFF * (grid[:, None] - cent) ** 2)
    Areg = Phi.T @ Phi + RIDGE * np.eye(NB)
    offs25 = np.linspace(0.0, CUTOFF, NG)
    ea_grid = np.exp(GCOEFF * (grid[:, None] - offs25) ** 2)
    alph = np.zeros((L, 128, FEAT), dtype=np.float32)
    for l in range(L):
        Y = (np.logaddexp(0.0, ea_grid @ w1[l] + b1[l]) - LOG2) @ w2[l] + b2[l]
        al = np.linalg.solve(Areg, Phi.T @ Y).astype(np.float32)   # [NB, FEAT]
        alph[l, 0:NB, :] = al
        alph[l, 64:64 + NB, :] = al

    l1w = np.asarray(lin1_w, dtype=np.float32)
    l2w = np.asarray(lin2_w, dtype=np.float32)
    l2b = np.asarray(lin2_b, dtype=np.float32)
    lw = np.asarray(lin_w, dtype=np.float32)
    lb = np.asarray(lin_b, dtype=np.float32)
    ow1 = np.asarray(out_w1, dtype=np.float32)
    ob1 = np.asarray(out_b1, dtype=np.float32)
    ow2 = np.asarray(out_w2, dtype=np.float32)
    ob2 = np.asarray(out_b2, dtype=np.float32)

    # diag tile holds 36.0 on the self-edge (d2m = max(min(d2,36), diag))
    diagbig = np.zeros((128, 8 * APM), dtype=np.float32)
    for p in range(128):
        for k in range(8):
            i = 8 * (p % 4) + k
            diagbig[p, 32 * k + i] = 36.0
    # exp(-8(d-c)^2)*gamma = Exp(16c*d + q - 8c^2): offs holds -8c^2 (Exp bias),
    # cmat the [16c; 1] coefficients for the K=2 broadcast matmul
    offs = np.zeros((128, 1), dtype=np.float32)
    cmat = np.zeros((2, 128), dtype=np.float32)
    for p in range(128):
        if p % 64 < NB:
            c = cent[p % 64]
            offs[p, 0] = GCOEFF * c * c
            cmat[0, p] = -2.0 * GCOEFF * c
            cmat[1, p] = 1.0

    shared = {
        "alph": alph.astype(BF),
        "l1w": np.ascontiguousarray(l1w),
        "l2w": np.ascontiguousarray(l2w),
        "l2b": np.ascontiguousarray(l2b.T),
        "lw": np.ascontiguousarray(lw),
        "lb": np.ascontiguousarray(lb.T),
        "ow1": np.ascontiguousarray(ow1),
        "ob1": np.ascontiguousarray(ob1),
        "ow2": np.ascontiguousarray(ow2),
        "ob2": ob2.reshape(1).copy(),
        "diagbig": diagbig.astype(BF),
        "offs": offs,
        "cmat": cmat,
    }
    in_maps = []
    for c in range(NCORES):
        sl = slice(NA * c, NA * (c + 1))
        h0 = emb[np.asarray(z[sl], dtype=np.int64)].T
        m = dict(shared)
        m["pos"] = pos[sl].copy()
        m["h0"] = np.ascontiguousarray(h0).astype(BF)
        in_maps.append(m)
    return in_maps


def kernel(**inputs) -> np.ndarray:
    from concourse.bass_utils import run_bass_kernel_spmd
    global _COMPILED
    if _COMPILED is None:
        _COMPILED = _build(1)
    nc = _COMPILED
    in_maps = _prep_inputs(**inputs)
    res = run_bass_kernel_spmd(nc, in_maps, list(range(NCORES)))
    out = np.concatenate([res.results[c]["energy"] for c in range(NCORES)])
    return out.astype(np.float32)


if __name__ == "__main__":
    _build(1)
    print("built ok")


# revision 52
# speedup vs baseline: 323.8057x; 1.0878x over previous
"""SchNet forward on 8 Trainium2 NeuronCores (Bass/Tile), data-parallel over molecules.

kernel(**inputs) takes FULL inputs (as produced by setup_inputs) and returns
the FULL [256] float32 per-molecule energies. Inside: shards 256 molecules
into 8 groups of 32 (1024 atoms each), runs an SPMD Bass kernel on cores 0-7,
gathers outputs.

Hardcoded shape: N=8192 atoms, 32 atoms/molecule, FEAT=100, NG=25, K=28, L=4,
CUTOFF=6.  Per core: 1024 atoms, all-pairs 32x32 block distances (E=32768
edge slots); top-28 selection via rank counting; non-selected edges get
distance=CUTOFF so the cosine cutoff zeroes them exactly like the reference's
top_k + ccut weighting.

v3 design notes:
 - repeats is a hardware For_i loop (NEFF size independent of repeat count).
 - atoms mapped p-major in phase A: atom a = 8p + k on partition p; molecule
   m = p//4; in-molecule index i = 8*(p%4) + k. Column c = k*32 + j, so a
   partition-major [128,256] -> DRAM write is exactly edge order e = a*32+j.
 - The edge filter W(d) = ssp(gauss(d) @ w1 + b1) @ w2 + b2 is a smooth scalar
   function of edge distance only, so it is refit host-side (ridge lstsq) onto
   a richer 51-Gaussian basis; the per-edge MLP collapses into one matmul with
   the fitted coefficients.  The cutoff weight gamma is folded into the basis
   values (and the fitted constant term rides on the Gaussians), so the whole
   per-edge filter application is: W*gamma = alpha^T @ (gauss51(d) * gamma).
 - basis values bf16, built once per rep; per-edge cutoff gamma broadcast via
   ones-vector matmuls (no 128-descriptor broadcast DMAs in inner loops).
"""

import math
import numpy as np

N = 8192
APM = 32
FEAT = 100
NG = 25
K = 28
L = 4
CUTOFF = 6.0
NCORES = 8
NA = N // NCORES          # atoms per core = 1024
NM = NA // APM            # molecules per core = 32
E = NA * APM              # edge slots per core = 32768
EG = E // 2               # edges per partition-group = 16384
EC = APM * APM            # edge chunk = one molecule's 32x32 pairs = 1024
H = FEAT // 2
NB = 51                   # fitted gaussian basis size
GCOEFF = -8.0             # gaussian exponent coefficient
RIDGE = 1e-4              # ridge strength for the filter refit
LOG2 = float(np.log(2.0))


_COMPILED = None


def _build(repeats: int = 1, hw_loop: bool = True, debug: bool = False,
           accum_out: bool = False):
    import contextlib
    import concourse.bass as bass
    import concourse.mybir as mybir
    import concourse.tile as tile
    from concourse import bacc

    dt = mybir.dt
    F32 = dt.float32
    BF16 = dt.bfloat16
    A = mybir.ActivationFunctionType
    OP = mybir.AluOpType
    AX = mybir.AxisListType
    LF = L * FEAT

    nc = bacc.Bacc()

    pos_d = nc.dram_tensor("pos", [NA, 3], F32, kind="ExternalInput")
    h0_d = nc.dram_tensor("h0", [FEAT, NA], BF16, kind="ExternalInput")
    alph_d = nc.dram_tensor("alph", [L, 128, FEAT], BF16, kind="ExternalInput")
    l1w_d = nc.dram_tensor("l1w", [L, FEAT, FEAT], F32, kind="ExternalInput")
    l2w_d = nc.dram_tensor("l2w", [L, FEAT, FEAT], F32, kind="ExternalInput")
    l2b_d = nc.dram_tensor("l2b", [FEAT, L], F32, kind="ExternalInput")
    lw_d = nc.dram_tensor("lw", [L, FEAT, FEAT], F32, kind="ExternalInput")
    lb_d = nc.dram_tensor("lb", [FEAT, L], F32, kind="ExternalInput")
    ow1_d = nc.dram_tensor("ow1", [FEAT, H], F32, kind="ExternalInput")
    ob1_d = nc.dram_tensor("ob1", [H], F32, kind="ExternalInput")
    ow2_d = nc.dram_tensor("ow2", [H, 1], F32, kind="ExternalInput")
    ob2_d = nc.dram_tensor("ob2", [1], F32, kind="ExternalInput")
    diag_d = nc.dram_tensor("diagbig", [128, 8 * APM], BF16, kind="ExternalInput")
    offs_d = nc.dram_tensor("offs", [128, 1], F32, kind="ExternalInput")
    cmat_d = nc.dram_tensor("cmat", [2, 128], F32, kind="ExternalInput")

    out_d = nc.dram_tensor("energy", [NM], F32, kind="ExternalOutput")

    dtil_dram = nc.dram_tensor("dtil_lin", [E], F32)
    qrow_dram = nc.dram_tensor("qrow_lin", [E], F32)
    if debug:
        dbg_ea = nc.dram_tensor("dbg_ea", [128, EG], F32, kind="ExternalOutput")
        dbg_x1 = nc.dram_tensor("dbg_x1", [FEAT, NA], F32, kind="ExternalOutput")
        dbg_agg = nc.dram_tensor("dbg_agg", [FEAT, NA], F32, kind="ExternalOutput")
        dbg_ps2 = nc.dram_tensor("dbg_ps2", [FEAT, EC], F32, kind="ExternalOutput")
        dbg_msg = nc.dram_tensor("dbg_msg", [FEAT, 2 * EC], F32,
                                 kind="ExternalOutput")
        dbg_al = nc.dram_tensor("dbg_al", [128, L * FEAT], F32,
                                kind="ExternalOutput")

    def bap(a, off, dims):
        return bass.AP(tensor=a.tensor, offset=a.offset + off, ap=dims)

    with tile.TileContext(nc) as tc:
        ctx = contextlib.ExitStack()
        with ctx:
            persist = ctx.enter_context(tc.tile_pool(name="persist", bufs=1))
            wpool = ctx.enter_context(tc.tile_pool(name="weights", bufs=1))
            psA = ctx.enter_context(tc.tile_pool(name="psA", bufs=2, space="PSUM"))
            psB = ctx.enter_context(tc.tile_pool(name="psB", bufs=2, space="PSUM"))

            # ---- persistent tiles
            ea_t = persist.tile([128, EG], BF16, tag="ea")  # 2 groups @ bases 0/64
            hA = persist.tile([FEAT, NA], F32, tag="hA")
            hB = persist.tile([FEAT, NA], F32, tag="hB")
            x1_t = persist.tile([FEAT, NA], F32, tag="x1")
            agg_t = persist.tile([FEAT, NA], F32, tag="agg")
            h0b_t = persist.tile([FEAT, NA], BF16, tag="h0b")
            nhalfpi_t = persist.tile([128, 1], F32, tag="nhalfpi")
            half_t = persist.tile([128, 1], F32, tag="half")
            diag16_t = persist.tile([128, 8 * APM], BF16, tag="diag16")
            diag_t = persist.tile([128, 8 * APM], F32, tag="diag")
            offs_t = persist.tile([128, 1], F32, tag="offs")
            cmat_t = persist.tile([2, 128], F32, tag="cmat")
            tiny_t = persist.tile([128, 1], F32, tag="tiny")
            nc.vector.memset(nhalfpi_t[:], -math.pi / 2)
            nc.vector.memset(half_t[:], 0.5)
            nc.vector.memset(tiny_t[:], 1e-30)
            nc.sync.dma_start(out=diag16_t[:], in_=diag_d[:])
            nc.sync.dma_start(out=offs_t[:], in_=offs_d[:])
            nc.sync.dma_start(out=cmat_t[:], in_=cmat_d[:])
            nc.scalar.activation(diag_t[:], diag16_t[:], A.Copy)

            # ---- weights
            alph_t = wpool.tile([128, LF], BF16, tag="alph")
            l1w_t = wpool.tile([FEAT, LF], F32, tag="l1w")
            l2w_t = wpool.tile([FEAT, LF], F32, tag="l2w")
            lw_t = wpool.tile([FEAT, LF], F32, tag="lww")
            l2b_t = wpool.tile([FEAT, L], F32, tag="l2b")
            lb_t = wpool.tile([FEAT, L], F32, tag="lb")
            ow1_t = wpool.tile([FEAT, H], F32, tag="ow1")
            ob1_t = wpool.tile([H, 1], F32, tag="ob1")
            ow2_t = wpool.tile([H, 1], F32, tag="ow2")
            ob2_t = wpool.tile([1, 1], F32, tag="ob2")

            nc.sync.dma_start(out=alph_t[:].rearrange("p (l f) -> p l f", f=FEAT),
                              in_=alph_d[:].transpose([1, 0, 2]))
            nc.sync.dma_start(out=l1w_t[:].rearrange("p (l f) -> p l f", f=FEAT),
                              in_=l1w_d[:].transpose([1, 0, 2]))
            nc.sync.dma_start(out=l2w_t[:].rearrange("p (l f) -> p l f", f=FEAT),
                              in_=l2w_d[:].transpose([1, 0, 2]))
            nc.sync.dma_start(out=lw_t[:].rearrange("p (l f) -> p l f", f=FEAT),
                              in_=lw_d[:].transpose([1, 0, 2]))
            nc.sync.dma_start(out=l2b_t[:], in_=l2b_d[:])
            nc.sync.dma_start(out=lb_t[:], in_=lb_d[:])
            nc.sync.dma_start(out=ow1_t[:], in_=ow1_d[:])
            nc.sync.dma_start(out=ob1_t[:], in_=ob1_d[:].unsqueeze(1))
            nc.sync.dma_start(out=ow2_t[:], in_=ow2_d[:])
            nc.sync.dma_start(out=ob2_t[:], in_=ob2_d[:].unsqueeze(1))

            if accum_out:
                acc_en = persist.tile([1, NM], F32, tag="acc_en")
                nc.vector.memset(acc_en[:], 0.0)
            loop_cm = (tc.For_i(0, repeats, 1) if hw_loop
                       else contextlib.nullcontext())
            with loop_cm:
                nc.sync.dma_start(out=h0b_t[:], in_=h0_d[:])
                nc.scalar.activation(hA[:], h0b_t[:], A.Copy)

                with tc.tile_pool(name="bld", bufs=1) as bp, \
                     tc.tile_pool(name="scrA", bufs=2) as sc:
                    # ========== PHASE A: graph build ==========
                    # atom a = 8p + k; molecule p//4; col c = 32k + j
                    posC = bp.tile([128, 8, 3], F32, tag="posC")
                    nc.sync.dma_start(
                        out=posC[:],
                        in_=bap(pos_d[:], 0, [[24, 128], [3, 8], [1, 3]]))
                    posB = bp.tile([128, APM, 3], F32, tag="posB")
                    nc.sync.dma_start(
                        out=posB[:],
                        in_=bap(pos_d[:], 0, [[96, 32], [0, 4], [1, 96]]))
                    dif = bp.tile([128, 8, APM, 3], F32, tag="dif")
                    pC = posC[:]
                    pB = posB[:]
                    nc.vector.tensor_tensor(
                        out=dif[:],
                        in0=bap(pC, 0, [pC.ap[0], [3, 8], [0, APM], [1, 3]]),
                        in1=bap(pB, 0, [pB.ap[0], [0, 8], [3, APM], [1, 3]]),
                        op=OP.subtract)
                    sq = bp.tile([128, 8, APM, 3], F32, tag="sq")
                    nc.scalar.activation(sq[:], dif[:], A.Square)
                    d2r = bp.tile([128, 8 * APM], F32, tag="d2r")
                    nc.vector.tensor_reduce(
                        out=d2r[:].rearrange("p (k j) -> p k j", j=APM),
                        in_=sq[:], axis=AX.X, op=OP.add)
                    # mask: diagonal + cutoff -> exactly 36 (diag tile holds 36)
                    d2m = bp.tile([128, 8 * APM], F32, tag="d2m")
                    nc.vector.scalar_tensor_tensor(
                        out=d2m[:], in0=d2r[:], scalar=36.0, in1=diag_t[:],
                        op0=OP.min, op1=OP.max)
                    # rank-based top-K selection (strict-less rank within molecule)
                    dd = d2m[:]
                    lt = bp.tile([128, 8, APM, APM], BF16, tag="lt")
                    nc.vector.tensor_tensor(
                        out=lt[:],
                        in0=bap(dd, 0, [dd.ap[0], [APM, 8], [0, APM], [1, APM]]),
                        in1=bap(dd, 0, [dd.ap[0], [APM, 8], [1, APM], [0, APM]]),
                        op=OP.is_lt)
                    rk = bp.tile([128, 8 * APM], F32, tag="rk")
                    nc.vector.tensor_reduce(
                        out=rk[:].rearrange("p (k j) -> p k j", j=APM),
                        in_=lt[:], axis=AX.X, op=OP.add)
                    sel = bp.tile([128, 8 * APM], F32, tag="sel")
                    nc.vector.tensor_scalar(out=sel[:], in0=rk[:],
                                            scalar1=float(K) - 0.5,
                                            scalar2=None, op0=OP.is_lt)
                    # dist = sqrt(d2m) with one Newton refinement
                    s_t = bp.tile([128, 8 * APM], F32, tag="s_t")
                    nc.scalar.activation(s_t[:], d2m[:], A.Sqrt)
                    rc = sc.tile([128, 8 * APM], F32, tag="rc")
                    nc.vector.reciprocal(rc[:], s_t[:])
                    tq = sc.tile([128, 8 * APM], F32, tag="tq")
                    nc.vector.tensor_tensor(out=tq[:], in0=d2m[:], in1=rc[:],
                                            op=OP.mult)
                    nc.vector.tensor_tensor(out=s_t[:], in0=s_t[:], in1=tq[:],
                                            op=OP.add)
                    # dtil = sel ? 0.5*s : 6
                    dtil = bp.tile([128, 8 * APM], F32, tag="dtil")
                    nc.vector.tensor_scalar(out=dtil[:], in0=s_t[:], scalar1=0.5,
                                            scalar2=-6.0, op0=OP.mult, op1=OP.add)
                    nc.vector.tensor_tensor(out=dtil[:], in0=dtil[:], in1=sel[:],
                                            op=OP.mult)
                    nc.vector.tensor_scalar(out=dtil[:], in0=dtil[:], scalar1=6.0,
                                            scalar2=None, op0=OP.add)
                    # gamma = 0.5*(cos(pi*d/6)+1) * sel * (d2m<36)
                    sn = bp.tile([128, 8 * APM], F32, tag="sn")
                    nc.scalar.activation(sn[:], dtil[:], A.Sin, bias=nhalfpi_t[:],
                                         scale=float(math.pi / 6.0))
                    nc.vector.tensor_scalar(out=sn[:], in0=sn[:], scalar1=-0.5,
                                            scalar2=0.5, op0=OP.mult, op1=OP.add)
                    ilt = bp.tile([128, 8 * APM], F32, tag="ilt")
                    nc.vector.scalar_tensor_tensor(
                        out=ilt[:], in0=d2m[:], scalar=36.0, in1=sel[:],
                        op0=OP.is_lt, op1=OP.mult)
                    gamf = bp.tile([128, 8 * APM], F32, tag="gamf")
                    nc.vector.tensor_tensor(out=gamf[:], in0=sn[:], in1=ilt[:],
                                            op=OP.mult)
                    # q = -8*d^2 + ln(gamma): basis exp(16c*d - 8c^2 + q)
                    gamln = bp.tile([128, 8 * APM], F32, tag="gamln")
                    nc.scalar.activation(gamln[:], gamf[:], A.Ln, bias=tiny_t[:])
                    t2 = sc.tile([128, 8 * APM], F32, tag="t2")
                    nc.scalar.activation(t2[:], dtil[:], A.Square)
                    qrow_t = bp.tile([128, 8 * APM], F32, tag="qrow")
                    nc.vector.scalar_tensor_tensor(
                        out=qrow_t[:], in0=t2[:], scalar=float(GCOEFF),
                        in1=gamln[:], op0=OP.mult, op1=OP.add)
                    # p-major col-contiguous writes == edge order e = a*32+j
                    nc.sync.dma_start(
                        out=bap(dtil_dram[:], 0, [[8 * APM, 128], [1, 8 * APM]]),
                        in_=dtil[:])
                    nc.sync.dma_start(
                        out=bap(qrow_dram[:], 0, [[8 * APM, 128], [1, 8 * APM]]),
                        in_=qrow_t[:])

                    # basis build: ea[b, e] = exp(-8*(d_e - cent_b)^2) * gamma_e
                    #            = Exp(16*c_b*d_e + q_e - 8*c_b^2)
                    # group g2 on partitions 64*g2 .. 64*g2+50
                    for s32 in range(16):
                        row2 = sc.tile([2, 2048], F32, tag="row2")
                        nc.sync.dma_start(
                            out=row2[0:1, :],
                            in_=dtil_dram[2048 * s32:2048 * (s32 + 1)].unsqueeze(0))
                        nc.sync.dma_start(
                            out=row2[1:2, :],
                            in_=qrow_dram[2048 * s32:2048 * (s32 + 1)].unsqueeze(0))
                        g2 = s32 // 8
                        rows = slice(64 * g2, 64 * g2 + NB)
                        for q in range(2):
                            psQ = psA.tile([128, EC], F32, tag="psA")
                            for h2 in range(2):
                                cs = 1024 * q + 512 * h2
                                nc.tensor.matmul(
                                    psQ[:, 512 * h2:512 * (h2 + 1)],
                                    cmat_t[:2, :],
                                    row2[:2, cs:cs + 512],
                                    start=True, stop=True)
                            cb = 2048 * (s32 % 8) + 1024 * q
                            nc.scalar.activation(ea_t[rows, cb:cb + EC],
                                                 psQ[rows, :], A.Exp,
                                                 bias=offs_t[rows, :])

                with tc.tile_pool(name="scrB", bufs=2) as sc:
                    # ========== PHASE B: interaction layers ==========
                    if debug:
                        dal = sc.tile([128, L * FEAT], F32, tag="dal")
                        nc.vector.tensor_copy(dal[:], alph_t[:])
                        nc.sync.dma_start(out=dbg_al[:], in_=dal[:])
                        for dc in range(8):
                            ds_ = slice(2048 * dc, 2048 * (dc + 1))
                            for g2 in range(2):
                                rows = slice(64 * g2, 64 * g2 + NB)
                                eaf = sc.tile([128, 2048], F32, tag="eaf")
                                nc.scalar.activation(eaf[rows, :],
                                                     ea_t[rows, ds_], A.Copy)
                                nc.sync.dma_start(out=dbg_ea[rows, ds_],
                                                  in_=eaf[rows, :])
                    hcur, hnxt = hA, hB
                    for l in range(L):
                        lf = slice(FEAT * l, FEAT * (l + 1))
                        ps_n = psA.tile([128, NA], F32, tag="psA")
                        for hh in range(2):
                            qs = slice(512 * hh, 512 * (hh + 1))
                            nc.tensor.matmul(ps_n[:FEAT, qs], l1w_t[:, lf],
                                             hcur[:, qs], start=True, stop=True)
                        nc.scalar.activation(x1_t[:], ps_n[:FEAT, :], A.Copy)

                        for mp in range(NM // 2):
                            msg = sc.tile([FEAT, 2 * EC], BF16, tag="msg")
                            for hf in range(2):
                                m = 2 * mp + hf
                                base = 64 * (m // 16)
                                cb = EC * (m % 16)
                                a0 = APM * m
                                ps2 = psB.tile([128, EC], F32, tag="psB")
                                for q2 in range(2):
                                    qs = slice(512 * q2, 512 * (q2 + 1))
                                    nc.tensor.matmul(
                                        ps2[:FEAT, qs],
                                        alph_t[base:base + NB, lf],
                                        ea_t[base:base + NB,
                                             cb + 512 * q2:cb + 512 * (q2 + 1)],
                                        start=True, stop=True)
                                x1b = x1_t[:]
                                if debug and l == 0 and mp == 0 and hf == 0:
                                    dps = sc.tile([FEAT, EC], F32, tag="dps")
                                    nc.vector.tensor_copy(dps[:], ps2[:FEAT, :])
                                    nc.sync.dma_start(out=dbg_ps2[:], in_=dps[:])
                                nc.vector.tensor_tensor(
                                    out=msg[:, EC * hf:EC * (hf + 1)],
                                    in0=ps2[:FEAT, :],
                                    in1=bap(x1b, a0,
                                            [x1b.ap[0], [0, APM], [1, APM]]),
                                    op=OP.mult)
                            if debug and l == 0 and mp == 0:
                                dmsg = sc.tile([FEAT, 2 * EC], F32, tag="dmsg")
                                nc.vector.tensor_copy(dmsg[:], msg[:])
                                nc.sync.dma_start(out=dbg_msg[:], in_=dmsg[:])
                            a0p = 2 * APM * mp
                            nc.vector.tensor_reduce(
                                out=agg_t[:, a0p:a0p + 2 * APM],
                                in_=msg[:].rearrange("p (a j) -> p a j", j=APM),
                                axis=AX.X, op=OP.add)

                        if debug and l == 0:
                            nc.sync.dma_start(out=dbg_x1[:], in_=x1_t[:])
                            nc.sync.dma_start(out=dbg_agg[:], in_=agg_t[:])
                        ps_v = psA.tile([128, NA], F32, tag="psA")
                        for hh in range(2):
                            qs = slice(512 * hh, 512 * (hh + 1))
                            nc.tensor.matmul(ps_v[:FEAT, qs], l2w_t[:, lf],
                                             agg_t[:, qs], start=True, stop=True)
                        spe = sc.tile([FEAT, NA], F32, tag="spe")
                        nc.scalar.activation(spe[:], ps_v[:FEAT, :], A.Exp,
                                             bias=l2b_t[:, l:l + 1])
                        spl = sc.tile([FEAT, NA], F32, tag="spl")
                        nc.scalar.activation(spl[:], spe[:], A.Ln,
                                             bias=half_t[:FEAT], scale=0.5)
                        ps_w = psB.tile([128, NA], F32, tag="psB")
                        for hh in range(2):
                            qs = slice(512 * hh, 512 * (hh + 1))
                            nc.tensor.matmul(ps_w[:FEAT, qs], lw_t[:, lf],
                                             spl[:, qs], start=True, stop=True)
                        nc.vector.scalar_tensor_tensor(
                            out=hnxt[:], in0=ps_w[:FEAT, :],
                            scalar=lb_t[:, l:l + 1],
                            in1=hcur[:], op0=OP.add, op1=OP.add)
                        hcur, hnxt = hnxt, hcur

                    # ========== PHASE C: readout ==========
                    ps_r = psA.tile([128, NA], F32, tag="psA")
                    for hh in range(2):
                        qs = slice(512 * hh, 512 * (hh + 1))
                        nc.tensor.matmul(ps_r[:H, qs], ow1_t[:], hcur[:, qs],
                                         start=True, stop=True)
                    ree = sc.tile([H, NA], F32, tag="spe")
                    nc.scalar.activation(ree[:], ps_r[:H, :], A.Exp,
                                         bias=ob1_t[:])
                    re = sc.tile([H, NA], F32, tag="spl")
                    nc.scalar.activation(re[:], ree[:], A.Ln,
                                         bias=half_t[:H], scale=0.5)
                    ps_e = psB.tile([128, NA], F32, tag="psB")
                    for hh in range(2):
                        qs = slice(512 * hh, 512 * (hh + 1))
                        nc.tensor.matmul(ps_e[:1, qs], ow2_t[:], re[:, qs],
                                         start=True, stop=True)
                    pa = sc.tile([1, NA], F32, tag="pa")
                    nc.vector.tensor_scalar(out=pa[:], in0=ps_e[:1, :],
                                            scalar1=ob2_t[:1, :], scalar2=None,
                                            op0=OP.add)
                    en = sc.tile([1, NM], F32, tag="en")
                    nc.vector.tensor_reduce(
                        out=en[:], in_=pa[:].rearrange("p (m i) -> p m i", i=APM),
                        axis=AX.X, op=OP.add)
                    if accum_out:
                        nc.vector.tensor_tensor(out=acc_en[:], in0=acc_en[:],
                                                in1=en[:], op=OP.add)
                        nc.sync.dma_start(out=out_d[:].unsqueeze(0),
                                          in_=acc_en[:])
                    else:
                        nc.sync.dma_start(out=out_d[:].unsqueeze(0), in_=en[:])

    nc.compile()
    return nc


def _prep_inputs(z, pos, ptr, emb, mlp_w1, mlp_b1, mlp_w2, mlp_b2,
                 lin1_w, lin2_w, lin2_b, lin_w, lin_b,
                 out_w1, out_b1, out_w2, out_b2):
    import ml_dtypes
    BF = ml_dtypes.bfloat16

    z = np.asarray(z)
    pos = np.ascontiguousarray(np.asarray(pos, dtype=np.float32))
    ptr = np.asarray(ptr)
    assert pos.shape == (N, 3)
    expect = np.arange(0, N + APM, APM)
    assert np.array_equal(ptr.astype(np.int64), expect), "non-uniform molecules unsupported"

    emb = np.asarray(emb, dtype=np.float32)
    w1 = np.asarray(mlp_w1, dtype=np.float64)        # [L, NG, FEAT]
    b1 = np.asarray(mlp_b1, dtype=np.float64)
    w2 = np.asarray(mlp_w2, dtype=np.float64)
    b2 = np.asarray(mlp_b2, dtype=np.float64)

    # refit the edge filter (a smooth function of distance) onto NB gaussians
    cent = np.linspace(0.0, CUTOFF, NB)
    grid = np.linspace(0.0, CUTOFF, 1201)
    Phi = np.exp(GCOEFF * (grid[:, None] - cent) ** 2)
    Areg = Phi.T @ Phi + RIDGE * np.eye(NB)
    offs25 = np.linspace(0.0, CUTOFF, NG)
    ea_grid = np.exp(GCOEFF * (grid[:, None] - offs25) ** 2)
    alph = np.zeros((L, 128, FEAT), dtype=np.float32)
    for l in range(L):
        Y = (np.logaddexp(0.0, ea_grid @ w1[l] + b1[l]) - LOG2) @ w2[l] + b2[l]
        al = np.linalg.solve(Areg, Phi.T @ Y).astype(np.float32)   # [NB, FEAT]
        alph[l, 0:NB, :] = al
        alph[l, 64:64 + NB, :] = al

    l1w = np.asarray(lin1_w, dtype=np.float32)
    l2w = np.asarray(lin2_w, dtype=np.float32)
    l2b = np.asarray(lin2_b, dtype=np.float32)
    lw = np.asarray(lin_w, dtype=np.float32)
    lb = np.asarray(lin_b, dtype=np.float32)
    ow1 = np.asarray(out_w1, dtype=np.float32)
    ob1 = np.asarray(out_b1, dtype=np.float32)
    ow2 = np.asarray(out_w2, dtype=np.float32)
    ob2 = np.asarray(out_b2, dtype=np.float32)

    # diag tile holds 36.0 on the self-edge (d2m = max(min(d2,36), diag))
    diagbig = np.zeros((128, 8 * APM), dtype=np.float32)
    for p in range(128):
        for k in range(8):
            i = 8 * (p % 4) + k
            diagbig[p, 32 * k + i] = 36.0
    # exp(-8(d-c)^2)*gamma = Exp(16c*d + q - 8c^2): offs holds -8c^2 (Exp bias),
    # cmat the [16c; 1] coefficients for the K=2 broadcast matmul
    offs = np.zeros((128, 1), dtype=np.float32)
    cmat = np.zeros((2, 128), dtype=np.float32)
    for p in range(128):
        if p % 64 < NB:
            c = cent[p % 64]
            offs[p, 0] = GCOEFF * c * c
            cmat[0, p] = -2.0 * GCOEFF * c
            cmat[1, p] = 1.0

    shared = {
        "alph": alph.astype(BF),
        "l1w": np.ascontiguousarray(l1w),
        "l2w": np.ascontiguousarray(l2w),
        "l2b": np.ascontiguousarray(l2b.T),
        "lw": np.ascontiguousarray(lw),
        "lb": np.ascontiguousarray(lb.T),
        "ow1": np.ascontiguousarray(ow1),
        "ob1": np.ascontiguousarray(ob1),
        "ow2": np.ascontiguousarray(ow2),
        "ob2": ob2.reshape(1).copy(),
        "diagbig": diagbig.astype(BF),
        "offs": offs,
        "cmat": cmat,
    }
    in_maps = []
    for c in range(NCORES):
        sl = slice(NA * c, NA * (c + 1))
        h0 = emb[np.asarray(z[sl], dtype=np.int64)].T
        m = dict(shared)
        m["pos"] = pos[sl].copy()
        m["h0"] = np.ascontiguousarray(h0).astype(BF)
        in_maps.append(m)
    return in_maps


def kernel(**inputs) -> np.ndarray:
    from concourse.bass_utils import run_bass_kernel_spmd
    global _COMPILED
    if _COMPILED is None:
        _COMPILED = _build(1)
    nc = _COMPILED
    in_maps = _prep_inputs(**inputs)
    res = run_bass_kernel_spmd(nc, in_maps, list(range(NCORES)))
    out = np.concatenate([res.results[c]["energy"] for c in range(NCORES)])
    return out.astype(np.float32)


if __name__ == "__main__":
    _build(1)
    print("built ok")
